# revision 1
# baseline (speedup 1.0000x reference)
# MoE layer (8 experts, top-2) on 8 TRN2 NeuronCores.
#
# Strategy: expert parallelism (core e owns expert e), per the sharding hint.
#   * Host (control plane): computes gate routing decisions, dispatches
#     ("all-to-all") each token's row to the core(s) owning its top-2 experts,
#     and combines the per-expert partial outputs back into the full output.
#   * Device (data plane): for each core e, computes
#         yT = sigmoid(dlg) * ( W2[e].T @ gelu( W1[e].T @ xT ) )
#     where xT is the (C x CAP) gathered token block for expert e (transposed
#     so the contraction dim lives on SBUF partitions), and sigmoid(dlg) is
#     exactly the top-2 softmax weight for the owning expert
#     (softmax([a,b])[0] == sigmoid(a-b)).
#
# Dataflow is fully transposed (features on partitions, tokens on the moving
# free dim) so neither matmul needs an intermediate transpose:
#     phase 1:  hT(F x T)  = W1.T @ xT   (accumulate over C tiles)  -> gelu
#     phase 2:  yT(C x T)  = W2.T @ hT   (accumulate over F tiles)  -> * ce
# W1 stays resident in SBUF in bf16; W2 streams per token block (its reloads
# hide under the PE-bound compute); tokens stream in blocks of 512 (the
# PSUM-bank moving-dim limit for fp32 accumulation).

import math

import numpy as np
import ml_dtypes

import concourse.bass as bass
import concourse.mybir as mybir
import concourse.tile as tile
from concourse import bacc
from concourse.bass_utils import run_bass_kernel_spmd

C = 1024          # d_model
F = 4096          # d_ff
E = 8             # experts == cores
P = 128           # SBUF partitions
NTOK = 512        # moving-dim token block (one PSUM bank of fp32)
BF16 = mybir.dt.bfloat16
F32 = mybir.dt.float32

# Filled by kernel() on each call, for the test harness to inspect.
last_run_info: dict = {}

# NEFF-module memo: cap -> compiled Bass module (routing is deterministic in
# the inputs, so repeat calls reuse the same module and its cached NEFF).
_nc_cache: dict = {}


def _build_ffn(cap: int, act_fn=None, ntok: int = NTOK) -> bass.Bass:
    """Per-core expert-FFN kernel: yt = sigmoid(dlg) * (w2.T @ gelu(w1.T @ xt))."""
    if act_fn is None:
        act_fn = mybir.ActivationFunctionType.Gelu
    nc = bacc.Bacc()
    CO = C // P   # 8 c-tiles
    FO = F // P   # 32 f-tiles

    xt = nc.dram_tensor("xt", [C, cap], BF16, kind="ExternalInput")
    w1 = nc.dram_tensor("w1", [C, F], BF16, kind="ExternalInput")
    # w2 is host-rearranged to [fi, co, fo, cc] so each (co) chunk streams as
    # one contiguous 8 KiB-per-partition DMA.
    w2 = nc.dram_tensor("w2", [P, CO, FO, P], BF16, kind="ExternalInput")
    dlg = nc.dram_tensor("dlg", [P, cap], F32, kind="ExternalInput")
    yt = nc.dram_tensor("yt", [C, cap], F32, kind="ExternalOutput")

    xt_r = xt.rearrange("(co ci) t -> ci co t", ci=P)
    yt_r = yt.rearrange("(co ci) t -> ci co t", ci=P)
    w1_r = w1.rearrange("(co ci) f -> ci co f", ci=P)

    with tile.TileContext(nc) as tc:
        with (
            tc.tile_pool(name="wts", bufs=1) as wpool,
            tc.tile_pool(name="w2s", bufs=3) as w2pool,
            tc.tile_pool(name="xts", bufs=2) as xpool,
            tc.tile_pool(name="hts", bufs=1) as hpool,
            tc.tile_pool(name="ces", bufs=2) as cepool,
            tc.tile_pool(name="yts", bufs=3) as ypool,
            tc.tile_pool(name="ps", bufs=4, space="PSUM") as pspool,
        ):
            # Block 0's token DMAs are issued BEFORE the w1 load: the DMA
            # queue is FIFO, and the first matmul needs xt — queueing 8 MiB
            # of w1 ahead of it costs a ~23 us PE ramp (measured in the
            # cost-model timeline).
            xt0 = xpool.tile([P, CO, ntok], BF16, tag="xt")
            nc.sync.dma_start(xt0[:, :, : min(ntok, cap)], xt_r[:, :, : min(ntok, cap)])

            # Resident w1 (bf16, 64 KiB/partition), loaded in f-major chunks
            # so phase 1's fo-th psum group only waits for the chunk covering
            # it, not the whole 8 MiB. w2 streams per token block.
            w1_sb = wpool.tile([P, CO, F], BF16, tag="w1")
            FCH = 1024
            for f0 in range(0, F, FCH):
                for co in range(CO):
                    nc.sync.dma_start(
                        w1_sb[:, co, f0 : f0 + FCH], w1_r[:, co, f0 : f0 + FCH]
                    )

            nblk = (cap + ntok - 1) // ntok
            for b in range(nblk):
                t0 = b * ntok
                tn = min(ntok, cap - t0)

                if b == 0:
                    xt_t = xt0
                else:
                    xt_t = xpool.tile([P, CO, ntok], BF16, tag="xt")
                    nc.sync.dma_start(xt_t[:, :, :tn], xt_r[:, :, t0 : t0 + tn])
                # Combine weight ce = sigmoid(dlg) = 0.5*tanh(dlg/2) + 0.5
                # (tanh shares an ACT table with gelu; sigmoid does not).
                # dlg isn't needed until phase 2, so even block 0's load sits
                # after the w1 chunks without stalling anything.
                dlg_t = cepool.tile([P, ntok], F32, tag="dlg")
                nc.sync.dma_start(dlg_t[:, :tn], dlg[:, t0 : t0 + tn])
                ce_t = cepool.tile([P, ntok], F32, tag="ce")
                nc.scalar.activation(
                    ce_t[:, :tn], dlg_t[:, :tn],
                    mybir.ActivationFunctionType.Tanh, scale=0.5,
                )
                nc.vector.tensor_scalar(
                    ce_t[:, :tn], ce_t[:, :tn], 0.5, 0.5,
                    mybir.AluOpType.mult, mybir.AluOpType.add,
                )

                # Phase 1: hT = gelu(W1.T @ xT) for this token block.
                ht_t = hpool.tile([P, FO, ntok], BF16, tag="ht")
                for fo in range(FO):
                    ps = pspool.tile([P, ntok], F32, tag="ps")
                    for co in range(CO):
                        nc.tensor.matmul(
                            ps[:, :tn],
                            w1_sb[:, co, fo * P : (fo + 1) * P],
                            xt_t[:, co, :tn],
                            start=(co == 0),
                            stop=(co == CO - 1),
                        )
                    nc.scalar.activation(ht_t[:, fo, :tn], ps[:, :tn], act_fn)

                # Phase 2: yT = ce * (W2.T @ hT).
                for co in range(CO):
                    w2_t = w2pool.tile([P, FO, P], BF16, tag="w2s")
                    nc.sync.dma_start(w2_t[:], w2[:, co, :, :])
                    ps2 = pspool.tile([P, ntok], F32, tag="ps")
                    for fo in range(FO):
                        nc.tensor.matmul(
                            ps2[:, :tn],
                            w2_t[:, fo, :],
                            ht_t[:, fo, :tn],
                            start=(fo == 0),
                            stop=(fo == FO - 1),
                        )
                    y_t = ypool.tile([P, ntok], F32, tag="y")
                    nc.vector.tensor_tensor(
                        y_t[:, :tn], ps2[:, :tn], ce_t[:, :tn],
                        mybir.AluOpType.mult,
                    )
                    nc.sync.dma_start(yt_r[:, co, t0 : t0 + tn], y_t[:, :tn])

    # bacc passes: register allocation, and crucially generate_event_semaphores,
    # which splits multi-wait sync conditions (HW allows 1 wait per instruction).
    nc.compile()

    # Guard: the Tile allocator believes SBUF is 224 KiB/partition (the ISA
    # constant), but exceeding ~192 KiB crashes the TRN2 exec unit. Keep a
    # hard ceiling so overflows fail at build time, not on silicon.
    hw = 0
    for alloc in nc.to_json()["functions"][0]["allocations"]:
        for ml in alloc.get("memorylocations") or []:
            if ml.get("type") == "SB":
                hw = max(hw, ml["addr"] + ml["dims"][1])
    assert hw <= 184 * 1024, f"SBUF high-water {hw / 1024:.1f} KiB exceeds 184 KiB"
    return nc


def _gate_jax_cpu(xf: np.ndarray, Wg: np.ndarray):
    """Reproduce the reference's gate bit-exactly: fp32 matmul + lax.top_k
    on the jax CPU backend (including its tie-breaking). Falls back to a
    numpy gate (correct except possibly on exact fp32 knife-edge ties) if
    jax is unavailable."""
    try:
        import jax

        cpu = jax.devices("cpu")[0]
        with jax.default_device(cpu):
            logits = jax.device_put(xf, cpu) @ jax.device_put(Wg, cpu)
            tv, ti = jax.lax.top_k(logits, 2)
            return np.asarray(ti), np.asarray(tv)
    except Exception:
        logits = xf @ Wg
        part = np.argpartition(-logits, 1, axis=1)[:, :2]
        pv = np.take_along_axis(logits, part, axis=1)
        order = np.argsort(-pv, axis=1, kind="stable")
        ti = np.take_along_axis(part, order, axis=1)
        tv = np.take_along_axis(logits, ti, axis=1)
        return ti, tv


def kernel(x, Wg, W1, W2):
    x = np.asarray(x, dtype=np.float32)
    Wg = np.asarray(Wg, dtype=np.float32)
    W1 = np.asarray(W1, dtype=np.float32)
    W2 = np.asarray(W2, dtype=np.float32)

    B, T, _ = x.shape
    N = B * T
    xf = x.reshape(N, C)

    # ---- Gate + routing (control plane) ----
    # Routing decisions are knife-edge sensitive: for this problem one token
    # has a 2.7e-7 gap between its 2nd and 3rd expert logits, smaller than
    # fp32 GEMM rounding differences between BLAS implementations. Compute
    # the gate with the same jax-on-CPU ops the reference uses so the top-2
    # selection matches it bit-for-bit.
    top2, tv = _gate_jax_cpu(xf, Wg)                        # (N, 2) ids / logits

    sels = []
    counts = []
    for e in range(E):
        sel = np.nonzero((top2 == e).any(axis=1))[0]
        sels.append(sel)
        counts.append(len(sel))
    # cap needs no partition alignment — tokens are the free dim everywhere.
    # Round to even so bf16 rows stay 4-byte aligned.
    cap = max(NTOK, math.ceil(max(counts) / 2) * 2)

    # ---- Token dispatch (all-to-all equivalent) ----
    in_maps = []
    for e in range(E):
        sel = sels[e]
        cnt = len(sel)
        row = top2[sel]
        tvr = tv[sel]
        own = np.where(row[:, 0] == e, tvr[:, 0], tvr[:, 1])
        other = np.where(row[:, 0] == e, tvr[:, 1], tvr[:, 0])

        xt = np.zeros((C, cap), dtype=ml_dtypes.bfloat16)
        xt[:, :cnt] = xf[sel].T.astype(ml_dtypes.bfloat16)
        dlg = np.full((cap,), -60.0, dtype=np.float32)
        dlg[:cnt] = own - other
        dlg_b = np.ascontiguousarray(
            np.broadcast_to(dlg[None, :], (P, cap)), dtype=np.float32
        )
        w2h = np.ascontiguousarray(
            W2[e].reshape(F // P, P, C // P, P).transpose(1, 2, 0, 3)
        ).astype(ml_dtypes.bfloat16)
        in_maps.append(
            {
                "xt": xt,
                "w1": W1[e].astype(ml_dtypes.bfloat16),
                "w2": w2h,
                "dlg": dlg_b,
            }
        )

    # ---- Expert FFN on the 8 NeuronCores ----
    nc = _nc_cache.get(cap)
    if nc is None:
        nc = _nc_cache[cap] = _build_ffn(cap)
    res = run_bass_kernel_spmd(nc, in_maps, core_ids=list(range(E)))

    global last_run_info
    last_run_info = {
        "cap": cap,
        "counts": counts,
        "exec_time_ns": res.exec_time_ns,
        "mean_exec_time_ns": res.mean_exec_time_ns,
        "instructions_and_trace": res.instructions_and_trace,
        "profile_json": res.profile_json,
    }

    # ---- Combine (weighted scatter-add) ----
    out = np.zeros((N, C), dtype=np.float32)
    for e in range(E):
        sel = sels[e]
        out[sel] += res.results[e]["yt"][:, : len(sel)].T
    return out.reshape(B, T, C)



# revision 2
# speedup vs baseline: 1.2812x; 1.2812x over previous
# MoE layer (8 experts, top-2) on 8 TRN2 NeuronCores.
#
# Strategy: expert parallelism (core e owns expert e), per the sharding hint.
#   * Host (control plane): computes gate routing decisions, dispatches
#     ("all-to-all") each token's row to the core(s) owning its top-2 experts,
#     and combines the per-expert partial outputs back into the full output.
#   * Device (data plane): for each core e, computes
#         yT = sigmoid(dlg) * ( W2[e].T @ gelu( W1[e].T @ xT ) )
#     in fp8 (e4m3) DoubleRow perf mode with hi/lo residual compensation.
#
# fp8 DoubleRow: one PE instruction computes lhsT[:,0].T @ rhs[:,0] +
# lhsT[:,1].T @ rhs[:,1] at 0.5 cycles per output row — two fp8 products for
# half the cost of one bf16 k-tile matmul. Every operand O is carried as
# O_hi = fp8(O), O_lo = fp8(O - O_hi), giving ~7 effective mantissa bits.
# Per k-tile pair (A, B) the product (Whi+Wlo)ᵀ(xhi+xlo) is computed as
# three DoubleRow instructions (the lo·lo term is dropped, ~0.07%):
#     main:   [Whi_A, Whi_B] · [xhi_A, xhi_B]
#     crossA: [Whi_A, Wlo_A] · [xlo_A, xhi_A]
#     crossB: [Whi_B, Wlo_B] · [xlo_B, xhi_B]
# = 0.75x the PE cycles of the bf16 kernel with ~0.8% end-to-end error.
#
# Weights are pre-scaled (s1=32, s2=64) so their values sit in e4m3's normal
# range; the scales are undone in the gelu input scale (1/s1) and the
# combine-weight multiply (ce/s2). h = gelu(pre) is split on-chip: the ACT
# engine writes gelu twice (fp8 h_hi and fp32 h), the DVE writes
# h_lo = fp8(h - h_hi).
#
# Dataflow is fully transposed (features on partitions, tokens on the moving
# free dim):
#     phase 1:  hT(F x T)  = W1.T @ xT   (accumulate over C tiles)  -> gelu
#     phase 2:  yT(C x T)  = W2.T @ hT   (accumulate over F tiles)  -> * ce
# W1 (hi+lo fp8, 64 KiB/partition) stays resident in SBUF; W2 streams per
# token block; tokens stream in blocks of 512.

import math

import numpy as np
import ml_dtypes

import concourse.bass as bass
import concourse.mybir as mybir
import concourse.tile as tile
from concourse import bacc
from concourse.bass_utils import run_bass_kernel_spmd

C = 1024          # d_model
F = 4096          # d_ff
E = 8             # experts == cores
P = 128           # SBUF partitions
NTOK = 512        # moving-dim token block (one PSUM bank of fp32)
S1 = 32.0         # W1 pre-scale (W1 ~ N(0, 1/C) -> ~N(0,1))
S2 = 64.0         # W2 pre-scale (W2 ~ N(0, 1/F) -> ~N(0,1))
F8 = mybir.dt.float8e4
F32 = mybir.dt.float32
E4 = ml_dtypes.float8_e4m3
DR = mybir.MatmulPerfMode.DoubleRow

# Filled by kernel() on each call, for the test harness to inspect.
last_run_info: dict = {}

# NEFF-module memo: cap -> compiled Bass module (routing is deterministic in
# the inputs, so repeat calls reuse the same module and its cached NEFF).
_nc_cache: dict = {}


def _build_ffn(cap: int, ntok: int = NTOK) -> bass.Bass:
    """Per-core expert-FFN kernel (fp8 DoubleRow, hi/lo compensated)."""
    nc = bacc.Bacc()
    CO = C // P   # 8 c-tiles
    FO = F // P   # 32 f-tiles

    # x8: [ci, co, slot, t] with slot 0 = lo, slot 1 = hi (cross pairing
    # needs rhs slots (lo, hi) against lhsT slots (hi, lo)).
    x8 = nc.dram_tensor("x8", [P, CO, 2, cap], F8, kind="ExternalInput")
    # w1: [ci, co, slot, f] with slot 0 = hi, slot 1 = lo.
    w1 = nc.dram_tensor("w1", [P, CO, 2, F], F8, kind="ExternalInput")
    # w2: [fi, co, fo, slot, cc] with slot 0 = hi, slot 1 = lo; each (co)
    # chunk streams as one contiguous 8 KiB-per-partition DMA.
    w2 = nc.dram_tensor("w2", [P, CO, FO, 2, P], F8, kind="ExternalInput")
    dlg = nc.dram_tensor("dlg", [P, cap], F32, kind="ExternalInput")
    yt = nc.dram_tensor("yt", [C, cap], F32, kind="ExternalOutput")

    yt_r = yt.rearrange("(co ci) t -> ci co t", ci=P)

    with tile.TileContext(nc) as tc:
        with (
            tc.tile_pool(name="wts", bufs=1) as wpool,
            tc.tile_pool(name="w2s", bufs=3) as w2pool,
            tc.tile_pool(name="xts", bufs=2) as xpool,
            tc.tile_pool(name="hts", bufs=1) as hpool,
            tc.tile_pool(name="hfs", bufs=4) as hfpool,
            tc.tile_pool(name="ces", bufs=2) as cepool,
            tc.tile_pool(name="yts", bufs=3) as ypool,
            tc.tile_pool(name="ps", bufs=4, space="PSUM") as pspool,
        ):
            # Block 0's token DMAs are issued BEFORE the w1 load: the DMA
            # queue is FIFO and the first matmul needs x8.
            xt0 = xpool.tile([P, CO, 2, ntok], F8, tag="xt")
            nc.sync.dma_start(
                xt0[:, :, :, : min(ntok, cap)], x8[:, :, :, : min(ntok, cap)]
            )

            # Resident W1 hi+lo (fp8, 64 KiB/partition), loaded in f-major
            # chunks so phase 1's fo-th psum group only waits for the chunk
            # covering it. W2 streams per token block.
            w1_sb = wpool.tile([P, CO, 2, F], F8, tag="w1")
            FCH = 1024
            for f0 in range(0, F, FCH):
                for co in range(CO):
                    nc.sync.dma_start(
                        w1_sb[:, co, :, f0 : f0 + FCH], w1[:, co, :, f0 : f0 + FCH]
                    )

            nblk = (cap + ntok - 1) // ntok
            for b in range(nblk):
                t0 = b * ntok
                tn = min(ntok, cap - t0)

                if b == 0:
                    xt_t = xt0
                else:
                    xt_t = xpool.tile([P, CO, 2, ntok], F8, tag="xt")
                    nc.sync.dma_start(xt_t[:, :, :, :tn], x8[:, :, :, t0 : t0 + tn])
                # Combine weight ce = sigmoid(dlg)/S2 = (0.5*tanh(dlg/2)+0.5)/S2
                # (tanh shares an ACT table with gelu; sigmoid does not). The
                # 1/S2 undoes the W2 pre-scale.
                dlg_t = cepool.tile([P, ntok], F32, tag="dlg")
                nc.sync.dma_start(dlg_t[:, :tn], dlg[:, t0 : t0 + tn])
                ce_t = cepool.tile([P, ntok], F32, tag="ce")
                nc.scalar.activation(
                    ce_t[:, :tn], dlg_t[:, :tn],
                    mybir.ActivationFunctionType.Tanh, scale=0.5,
                )
                nc.vector.tensor_scalar(
                    ce_t[:, :tn], ce_t[:, :tn], 0.5 / S2, 0.5 / S2,
                    mybir.AluOpType.mult, mybir.AluOpType.add,
                )

                # Phase 1: hT = gelu(W1.T @ xT) for this token block, split
                # into fp8 hi/lo planes. psum accumulates S1 * pre-act.
                ht_t = hpool.tile([P, FO, 2, ntok], F8, tag="ht")
                for fo in range(FO):
                    fsl = slice(fo * P, (fo + 1) * P)
                    ps = pspool.tile([P, ntok], F32, tag="ps")
                    for j in range(CO // 2):
                        nc.tensor.matmul(
                            ps[:, :tn],
                            w1_sb[:, 2 * j : 2 * j + 2, 0, fsl],
                            xt_t[:, 2 * j : 2 * j + 2, 1, :tn],
                            start=(j == 0), stop=False, perf_mode=DR,
                        )
                    for co in range(CO):
                        nc.tensor.matmul(
                            ps[:, :tn],
                            w1_sb[:, co, :, fsl],
                            xt_t[:, co, :, :tn],
                            start=False, stop=(co == CO - 1), perf_mode=DR,
                        )
                    # h_hi = fp8(gelu(ps/S1)); h = fp32 gelu (same ACT table,
                    # bit-identical inputs); h_lo = fp8(h - h_hi).
                    nc.scalar.activation(
                        ht_t[:, fo, 1, :tn], ps[:, :tn],
                        mybir.ActivationFunctionType.Gelu, scale=1.0 / S1,
                    )
                    hf_t = hfpool.tile([P, ntok], F32, tag="hf")
                    nc.scalar.activation(
                        hf_t[:, :tn], ps[:, :tn],
                        mybir.ActivationFunctionType.Gelu, scale=1.0 / S1,
                    )
                    nc.vector.tensor_tensor(
                        ht_t[:, fo, 0, :tn], hf_t[:, :tn], ht_t[:, fo, 1, :tn],
                        mybir.AluOpType.subtract,
                    )

                # Phase 2: yT = (ce/S2) * (S2 * W2.T @ hT).
                for co in range(CO):
                    w2_t = w2pool.tile([P, FO, 2, P], F8, tag="w2s")
                    nc.sync.dma_start(w2_t[:], w2[:, co, :, :, :])
                    ps2 = pspool.tile([P, ntok], F32, tag="ps")
                    for j in range(FO // 2):
                        nc.tensor.matmul(
                            ps2[:, :tn],
                            w2_t[:, 2 * j : 2 * j + 2, 0, :],
                            ht_t[:, 2 * j : 2 * j + 2, 1, :tn],
                            start=(j == 0), stop=False, perf_mode=DR,
                        )
                    for fo in range(FO):
                        nc.tensor.matmul(
                            ps2[:, :tn],
                            w2_t[:, fo, :, :],
                            ht_t[:, fo, :, :tn],
                            start=False, stop=(fo == FO - 1), perf_mode=DR,
                        )
                    y_t = ypool.tile([P, ntok], F32, tag="y")
                    nc.vector.tensor_tensor(
                        y_t[:, :tn], ps2[:, :tn], ce_t[:, :tn],
                        mybir.AluOpType.mult,
                    )
                    nc.sync.dma_start(yt_r[:, co, t0 : t0 + tn], y_t[:, :tn])

    # bacc passes: register allocation, and crucially generate_event_semaphores,
    # which splits multi-wait sync conditions (HW allows 1 wait per instruction).
    nc.compile()

    # Guard: the Tile allocator believes SBUF is 224 KiB/partition (the ISA
    # constant), but exceeding ~192 KiB crashes the TRN2 exec unit. Keep a
    # hard ceiling so overflows fail at build time, not on silicon.
    hw = 0
    for alloc in nc.to_json()["functions"][0]["allocations"]:
        for ml in alloc.get("memorylocations") or []:
            if ml.get("type") == "SB":
                hw = max(hw, ml["addr"] + ml["dims"][1])
    assert hw <= 184 * 1024, f"SBUF high-water {hw / 1024:.1f} KiB exceeds 184 KiB"
    return nc


def _gate_jax_cpu(xf: np.ndarray, Wg: np.ndarray):
    """Reproduce the reference's gate bit-exactly: fp32 matmul + lax.top_k
    on the jax CPU backend (including its tie-breaking). Falls back to a
    numpy gate (correct except possibly on exact fp32 knife-edge ties) if
    jax is unavailable."""
    try:
        import jax

        cpu = jax.devices("cpu")[0]
        with jax.default_device(cpu):
            logits = jax.device_put(xf, cpu) @ jax.device_put(Wg, cpu)
            tv, ti = jax.lax.top_k(logits, 2)
            return np.asarray(ti), np.asarray(tv)
    except Exception:
        logits = xf @ Wg
        part = np.argpartition(-logits, 1, axis=1)[:, :2]
        pv = np.take_along_axis(logits, part, axis=1)
        order = np.argsort(-pv, axis=1, kind="stable")
        ti = np.take_along_axis(part, order, axis=1)
        tv = np.take_along_axis(logits, ti, axis=1)
        return ti, tv


def _split8(a: np.ndarray):
    """hi/lo e4m3 residual split of a float32 array."""
    hi = a.astype(E4)
    lo = (a - hi.astype(np.float32)).astype(E4)
    return hi, lo


def kernel(x, Wg, W1, W2):
    x = np.asarray(x, dtype=np.float32)
    Wg = np.asarray(Wg, dtype=np.float32)
    W1 = np.asarray(W1, dtype=np.float32)
    W2 = np.asarray(W2, dtype=np.float32)

    B, T, _ = x.shape
    N = B * T
    xf = x.reshape(N, C)
    CO, FO = C // P, F // P

    # ---- Gate + routing (control plane) ----
    # Routing decisions are knife-edge sensitive: compute the gate with the
    # same jax-on-CPU ops the reference uses so the top-2 selection matches
    # it bit-for-bit.
    top2, tv = _gate_jax_cpu(xf, Wg)                        # (N, 2) ids / logits

    sels = []
    counts = []
    for e in range(E):
        sel = np.nonzero((top2 == e).any(axis=1))[0]
        sels.append(sel)
        counts.append(len(sel))
    # cap needs no partition alignment — tokens are the free dim everywhere.
    # Round to even so the DoubleRow moving pair stays aligned.
    cap = max(NTOK, math.ceil(max(counts) / 2) * 2)

    # ---- Token dispatch (all-to-all equivalent) ----
    in_maps = []
    for e in range(E):
        sel = sels[e]
        cnt = len(sel)
        row = top2[sel]
        tvr = tv[sel]
        own = np.where(row[:, 0] == e, tvr[:, 0], tvr[:, 1])
        other = np.where(row[:, 0] == e, tvr[:, 1], tvr[:, 0])

        # x8: [ci, co, slot(0=lo,1=hi), t]
        xe = np.zeros((P, CO, 2, cap), dtype=E4)
        xt = xf[sel].T.reshape(CO, P, cnt).transpose(1, 0, 2)  # (ci, co, t)
        xhi, xlo = _split8(xt)
        xe[:, :, 0, :cnt] = xlo
        xe[:, :, 1, :cnt] = xhi

        dlg_v = np.full((cap,), -60.0, dtype=np.float32)
        dlg_v[:cnt] = own - other
        dlg_b = np.ascontiguousarray(
            np.broadcast_to(dlg_v[None, :], (P, cap)), dtype=np.float32
        )

        # w1: [ci, co, slot(0=hi,1=lo), f]
        w1t = (W1[e] * S1).reshape(CO, P, F).transpose(1, 0, 2)  # (ci, co, f)
        w1hi, w1lo = _split8(w1t)
        w1e = np.stack([w1hi, w1lo], axis=2)

        # w2: [fi, co, fo, slot(0=hi,1=lo), cc]
        w2t = (W2[e] * S2).reshape(FO, P, CO, P).transpose(1, 2, 0, 3)
        w2hi, w2lo = _split8(w2t)                      # (fi, co, fo, cc)
        w2e = np.stack([w2hi, w2lo], axis=3)

        in_maps.append(
            {
                "x8": np.ascontiguousarray(xe),
                "w1": np.ascontiguousarray(w1e),
                "w2": np.ascontiguousarray(w2e),
                "dlg": dlg_b,
            }
        )

    # ---- Expert FFN on the 8 NeuronCores ----
    nc = _nc_cache.get(cap)
    if nc is None:
        nc = _nc_cache[cap] = _build_ffn(cap)
    res = run_bass_kernel_spmd(nc, in_maps, core_ids=list(range(E)))

    global last_run_info
    last_run_info = {
        "cap": cap,
        "counts": counts,
        "exec_time_ns": res.exec_time_ns,
        "mean_exec_time_ns": res.mean_exec_time_ns,
        "instructions_and_trace": res.instructions_and_trace,
        "profile_json": res.profile_json,
    }

    # ---- Combine (weighted scatter-add) ----
    out = np.zeros((N, C), dtype=np.float32)
    for e in range(E):
        sel = sels[e]
        out[sel] += res.results[e]["yt"][:, : len(sel)].T
    return out.reshape(B, T, C)


# revision 7
# speedup vs baseline: 1.4164x; 1.1055x over previous
# MoE layer (8 experts, top-2) on 8 TRN2 NeuronCores.
#
# Strategy: expert parallelism (core e owns expert e), per the sharding hint.
#   * Host (control plane): computes gate routing decisions, dispatches
#     ("all-to-all") each token's row to the core(s) owning its top-2 experts,
#     and combines the per-expert partial outputs back into the full output.
#   * Device (data plane): for each core e, computes
#         yT = sigmoid(dlg) * ( W2[e].T @ gelu( W1[e].T @ xT ) )
#     in fp8 (e4m3) DoubleRow perf mode with hi/lo residual compensation.
#
# fp8 DoubleRow: one PE instruction computes lhsT[:,0].T @ rhs[:,0] +
# lhsT[:,1].T @ rhs[:,1] at 0.5 cycles per output row — two fp8 products for
# half the cost of one bf16 k-tile matmul. Every operand O is carried as
# O_hi = fp8(O), O_lo = fp8(O - O_hi), giving ~7 effective mantissa bits.
# Per k-tile pair (A, B) the product (Whi+Wlo)ᵀ(xhi+xlo) is computed as
# three DoubleRow instructions (the lo·lo term is dropped, ~0.07%):
#     main:   [Whi_A, Whi_B] · [xhi_A, xhi_B]
#     crossA: [Whi_A, Wlo_A] · [xlo_A, xhi_A]
#     crossB: [Whi_B, Wlo_B] · [xlo_B, xhi_B]
# = 0.75x the PE cycles of the bf16 kernel with ~0.2% end-to-end error.
#
# Each core's tokens are sorted by their combine weight (descending), and the
# LAST 3 of the 9 token blocks — the ~38% of token-paths with the smallest
# combine weights — drop the x/h (activation) compensation, keeping only the
# weight compensation (main + W_lo cross = 2 products per k-tile, 0.5x bf16).
# Their larger path error (~3.7%) is scaled by small combine weights; the
# measured end-to-end error is ~1.2e-2 against the 2e-2 gate (the numpy
# scheme simulator reproduces the hardware error to 3 decimal places).
#
# Weights are pre-scaled (s1=32, s2=64) so their values sit in e4m3's normal
# range; the scales are undone in the gelu input scale (1/s1) and the
# combine-weight multiply (ce/s2). h = gelu(pre) is split on-chip: the ACT
# engine writes gelu twice (fp8 h_hi and fp32 h), the DVE writes
# h_lo = fp8(h - h_hi).
#
# Dataflow is fully transposed (features on partitions, tokens on the moving
# free dim):
#     phase 1:  hT(F x T)  = W1.T @ xT   (accumulate over C tiles)  -> gelu
#     phase 2:  yT(C x T)  = W2.T @ hT   (accumulate over F tiles)  -> * ce
# W1 (hi+lo fp8, 64 KiB/partition) stays resident in SBUF; W2 streams per
# token block; tokens stream in blocks of 512.

import math

import numpy as np
import ml_dtypes

import concourse.bass as bass
import concourse.mybir as mybir
import concourse.tile as tile
from concourse import bacc
from concourse.bass_utils import run_bass_kernel_spmd

C = 1024          # d_model
F = 4096          # d_ff
E = 8             # experts == cores
P = 128           # SBUF partitions
NTOK = 512        # moving-dim token block (one PSUM bank of fp32)
S1 = 32.0         # W1 pre-scale (W1 ~ N(0, 1/C) -> ~N(0,1))
S2 = 64.0         # W2 pre-scale (W2 ~ N(0, 1/F) -> ~N(0,1))
F8 = mybir.dt.float8e4
F32 = mybir.dt.float32
E4 = ml_dtypes.float8_e4m3
DR = mybir.MatmulPerfMode.DoubleRow

# Filled by kernel() on each call, for the test harness to inspect.
last_run_info: dict = {}

# NEFF-module memo: cap -> compiled Bass module (routing is deterministic in
# the inputs, so repeat calls reuse the same module and its cached NEFF).
_nc_cache: dict = {}


N_WONLY = 3       # trailing token blocks that drop x/h compensation


def _block_full(b: int, nblk: int) -> bool:
    """True if block b keeps full (x and h) compensation."""
    return b < nblk - N_WONLY


def _build_ffn(cap: int, ntok: int = NTOK) -> bass.Bass:
    """Per-core expert-FFN kernel (fp8 DoubleRow, hi/lo compensated)."""
    nc = bacc.Bacc()
    CO = C // P   # 8 c-tiles
    FO = F // P   # 32 f-tiles

    # x8: [ci, co, slot, t] with slot 0 = lo, slot 1 = hi (cross pairing
    # needs rhs slots (lo, hi) against lhsT slots (hi, lo)).
    x8 = nc.dram_tensor("x8", [P, CO, 2, cap], F8, kind="ExternalInput")
    # w1: [ci, co, slot, f] with slot 0 = hi, slot 1 = lo.
    w1 = nc.dram_tensor("w1", [P, CO, 2, F], F8, kind="ExternalInput")
    # w2: [fi, co, fo, slot, cc] with slot 0 = hi, slot 1 = lo; each (co)
    # chunk streams as one contiguous 8 KiB-per-partition DMA.
    w2 = nc.dram_tensor("w2", [P, CO, FO, 2, P], F8, kind="ExternalInput")
    dlg = nc.dram_tensor("dlg", [P, cap], F32, kind="ExternalInput")
    yt = nc.dram_tensor("yt", [C, cap], F32, kind="ExternalOutput")

    yt_r = yt.rearrange("(co ci) t -> ci co t", ci=P)

    with tile.TileContext(nc) as tc:
        with (
            tc.tile_pool(name="wts", bufs=1) as wpool,
            tc.tile_pool(name="w2s", bufs=3) as w2pool,
            tc.tile_pool(name="xts", bufs=2) as xpool,
            tc.tile_pool(name="hts", bufs=1) as hpool,
            tc.tile_pool(name="hfs", bufs=4) as hfpool,
            tc.tile_pool(name="ces", bufs=2) as cepool,
            tc.tile_pool(name="yts", bufs=3) as ypool,
            tc.tile_pool(name="ps", bufs=4, space="PSUM") as pspool,
        ):
            # Block 0's token DMAs are issued BEFORE the w1 load: the DMA
            # queue is FIFO and the first matmul needs x8.
            xt0 = xpool.tile([P, CO, 2, ntok], F8, tag="xt")
            nc.sync.dma_start(
                xt0[:, :, :, : min(ntok, cap)], x8[:, :, :, : min(ntok, cap)]
            )

            # Resident W1 hi+lo (fp8, 64 KiB/partition), loaded in f-major
            # chunks so phase 1's fo-th psum group only waits for the chunk
            # covering it. W2 streams per token block.
            w1_sb = wpool.tile([P, CO, 2, F], F8, tag="w1")
            FCH = 1024
            for f0 in range(0, F, FCH):
                for co in range(CO):
                    nc.sync.dma_start(
                        w1_sb[:, co, :, f0 : f0 + FCH], w1[:, co, :, f0 : f0 + FCH]
                    )

            nblk = (cap + ntok - 1) // ntok
            for b in range(nblk):
                t0 = b * ntok
                tn = min(ntok, cap - t0)
                full = _block_full(b, nblk)

                if b == 0:
                    xt_t = xt0
                else:
                    xt_t = xpool.tile([P, CO, 2, ntok], F8, tag="xt")
                    if full:
                        nc.sync.dma_start(
                            xt_t[:, :, :, :tn], x8[:, :, :, t0 : t0 + tn]
                        )
                    else:
                        # W-only blocks never read the x_lo plane.
                        nc.sync.dma_start(
                            xt_t[:, :, 1, :tn], x8[:, :, 1, t0 : t0 + tn]
                        )
                # Combine weight ce = sigmoid(dlg)/S2 = (0.5*tanh(dlg/2)+0.5)/S2
                # (tanh shares an ACT table with gelu; sigmoid does not). The
                # 1/S2 undoes the W2 pre-scale.
                dlg_t = cepool.tile([P, ntok], F32, tag="dlg")
                nc.sync.dma_start(dlg_t[:, :tn], dlg[:, t0 : t0 + tn])
                ce_t = cepool.tile([P, ntok], F32, tag="ce")
                nc.scalar.activation(
                    ce_t[:, :tn], dlg_t[:, :tn],
                    mybir.ActivationFunctionType.Tanh, scale=0.5,
                )
                nc.vector.tensor_scalar(
                    ce_t[:, :tn], ce_t[:, :tn], 0.5 / S2, 0.5 / S2,
                    mybir.AluOpType.mult, mybir.AluOpType.add,
                )

                # Phase 1: hT = gelu(W1.T @ xT) for this token block, split
                # into fp8 hi/lo planes (hi only for W-only blocks). psum
                # accumulates S1 * pre-act.
                ht_t = hpool.tile([P, FO, 2, ntok], F8, tag="ht")
                for fo in range(FO):
                    fsl = slice(fo * P, (fo + 1) * P)
                    ps = pspool.tile([P, ntok], F32, tag="ps")
                    for j in range(CO // 2):
                        nc.tensor.matmul(
                            ps[:, :tn],
                            w1_sb[:, 2 * j : 2 * j + 2, 0, fsl],
                            xt_t[:, 2 * j : 2 * j + 2, 1, :tn],
                            start=(j == 0), stop=False, perf_mode=DR,
                        )
                    if full:
                        for co in range(CO):
                            nc.tensor.matmul(
                                ps[:, :tn],
                                w1_sb[:, co, :, fsl],
                                xt_t[:, co, :, :tn],
                                start=False, stop=(co == CO - 1), perf_mode=DR,
                            )
                    else:
                        for j in range(CO // 2):
                            nc.tensor.matmul(
                                ps[:, :tn],
                                w1_sb[:, 2 * j : 2 * j + 2, 1, fsl],
                                xt_t[:, 2 * j : 2 * j + 2, 1, :tn],
                                start=False, stop=(j == CO // 2 - 1),
                                perf_mode=DR,
                            )
                    # h_hi = fp8(gelu(ps/S1)); h = fp32 gelu (same ACT table,
                    # bit-identical inputs); h_lo = fp8(h - h_hi).
                    nc.scalar.activation(
                        ht_t[:, fo, 1, :tn], ps[:, :tn],
                        mybir.ActivationFunctionType.Gelu, scale=1.0 / S1,
                    )
                    if full:
                        hf_t = hfpool.tile([P, ntok], F32, tag="hf")
                        nc.scalar.activation(
                            hf_t[:, :tn], ps[:, :tn],
                            mybir.ActivationFunctionType.Gelu, scale=1.0 / S1,
                        )
                        nc.vector.tensor_tensor(
                            ht_t[:, fo, 0, :tn], hf_t[:, :tn],
                            ht_t[:, fo, 1, :tn], mybir.AluOpType.subtract,
                        )

                # Phase 2: yT = (ce/S2) * (S2 * W2.T @ hT).
                for co in range(CO):
                    w2_t = w2pool.tile([P, FO, 2, P], F8, tag="w2s")
                    nc.sync.dma_start(w2_t[:], w2[:, co, :, :, :])
                    ps2 = pspool.tile([P, ntok], F32, tag="ps")
                    for j in range(FO // 2):
                        nc.tensor.matmul(
                            ps2[:, :tn],
                            w2_t[:, 2 * j : 2 * j + 2, 0, :],
                            ht_t[:, 2 * j : 2 * j + 2, 1, :tn],
                            start=(j == 0), stop=False, perf_mode=DR,
                        )
                    if full:
                        for fo in range(FO):
                            nc.tensor.matmul(
                                ps2[:, :tn],
                                w2_t[:, fo, :, :],
                                ht_t[:, fo, :, :tn],
                                start=False, stop=(fo == FO - 1), perf_mode=DR,
                            )
                    else:
                        for j in range(FO // 2):
                            nc.tensor.matmul(
                                ps2[:, :tn],
                                w2_t[:, 2 * j : 2 * j + 2, 1, :],
                                ht_t[:, 2 * j : 2 * j + 2, 1, :tn],
                                start=False, stop=(j == FO // 2 - 1),
                                perf_mode=DR,
                            )
                    y_t = ypool.tile([P, ntok], F32, tag="y")
                    nc.vector.tensor_tensor(
                        y_t[:, :tn], ps2[:, :tn], ce_t[:, :tn],
                        mybir.AluOpType.mult,
                    )
                    nc.sync.dma_start(yt_r[:, co, t0 : t0 + tn], y_t[:, :tn])

    # bacc passes: register allocation, and crucially generate_event_semaphores,
    # which splits multi-wait sync conditions (HW allows 1 wait per instruction).
    nc.compile()

    # Guard: the Tile allocator believes SBUF is 224 KiB/partition (the ISA
    # constant), but exceeding ~192 KiB crashes the TRN2 exec unit. Keep a
    # hard ceiling so overflows fail at build time, not on silicon.
    hw = 0
    for alloc in nc.to_json()["functions"][0]["allocations"]:
        for ml in alloc.get("memorylocations") or []:
            if ml.get("type") == "SB":
                hw = max(hw, ml["addr"] + ml["dims"][1])
    assert hw <= 184 * 1024, f"SBUF high-water {hw / 1024:.1f} KiB exceeds 184 KiB"
    return nc


def _gate_jax_cpu(xf: np.ndarray, Wg: np.ndarray):
    """Reproduce the reference's gate bit-exactly: fp32 matmul + lax.top_k
    on the jax CPU backend (including its tie-breaking). Falls back to a
    numpy gate (correct except possibly on exact fp32 knife-edge ties) if
    jax is unavailable."""
    try:
        import jax

        cpu = jax.devices("cpu")[0]
        with jax.default_device(cpu):
            logits = jax.device_put(xf, cpu) @ jax.device_put(Wg, cpu)
            tv, ti = jax.lax.top_k(logits, 2)
            return np.asarray(ti), np.asarray(tv)
    except Exception:
        logits = xf @ Wg
        part = np.argpartition(-logits, 1, axis=1)[:, :2]
        pv = np.take_along_axis(logits, part, axis=1)
        order = np.argsort(-pv, axis=1, kind="stable")
        ti = np.take_along_axis(part, order, axis=1)
        tv = np.take_along_axis(logits, ti, axis=1)
        return ti, tv


def _split8(a: np.ndarray):
    """hi/lo e4m3 residual split of a float32 array."""
    hi = a.astype(E4)
    lo = (a - hi.astype(np.float32)).astype(E4)
    return hi, lo


def kernel(x, Wg, W1, W2):
    x = np.asarray(x, dtype=np.float32)
    Wg = np.asarray(Wg, dtype=np.float32)
    W1 = np.asarray(W1, dtype=np.float32)
    W2 = np.asarray(W2, dtype=np.float32)

    B, T, _ = x.shape
    N = B * T
    xf = x.reshape(N, C)
    CO, FO = C // P, F // P

    # ---- Gate + routing (control plane) ----
    # Routing decisions are knife-edge sensitive: compute the gate with the
    # same jax-on-CPU ops the reference uses so the top-2 selection matches
    # it bit-for-bit.
    top2, tv = _gate_jax_cpu(xf, Wg)                        # (N, 2) ids / logits

    # Softmax weights for the sort: own = weight of the owning expert.
    wsm = np.exp(tv - tv.max(1, keepdims=True))
    wsm = wsm / wsm.sum(1, keepdims=True)

    sels = []
    counts = []
    for e in range(E):
        sel = np.nonzero((top2 == e).any(axis=1))[0]
        # Sort descending by this expert's combine weight so the trailing
        # (W-only compensated) blocks hold the lowest-stakes token paths.
        own_w = np.where(top2[sel, 0] == e, wsm[sel, 0], wsm[sel, 1])
        sel = sel[np.argsort(-own_w, kind="stable")]
        sels.append(sel)
        counts.append(len(sel))
    # cap needs no partition alignment — tokens are the free dim everywhere.
    # Round to even so the DoubleRow moving pair stays aligned.
    cap = max(NTOK, math.ceil(max(counts) / 2) * 2)

    # ---- Token dispatch (all-to-all equivalent) ----
    in_maps = []
    for e in range(E):
        sel = sels[e]
        cnt = len(sel)
        row = top2[sel]
        tvr = tv[sel]
        own = np.where(row[:, 0] == e, tvr[:, 0], tvr[:, 1])
        other = np.where(row[:, 0] == e, tvr[:, 1], tvr[:, 0])

        # x8: [ci, co, slot(0=lo,1=hi), t]
        xe = np.zeros((P, CO, 2, cap), dtype=E4)
        xt = xf[sel].T.reshape(CO, P, cnt).transpose(1, 0, 2)  # (ci, co, t)
        xhi, xlo = _split8(xt)
        xe[:, :, 0, :cnt] = xlo
        xe[:, :, 1, :cnt] = xhi

        dlg_v = np.full((cap,), -60.0, dtype=np.float32)
        dlg_v[:cnt] = own - other
        dlg_b = np.ascontiguousarray(
            np.broadcast_to(dlg_v[None, :], (P, cap)), dtype=np.float32
        )

        # w1: [ci, co, slot(0=hi,1=lo), f]
        w1t = (W1[e] * S1).reshape(CO, P, F).transpose(1, 0, 2)  # (ci, co, f)
        w1hi, w1lo = _split8(w1t)
        w1e = np.stack([w1hi, w1lo], axis=2)

        # w2: [fi, co, fo, slot(0=hi,1=lo), cc]
        w2t = (W2[e] * S2).reshape(FO, P, CO, P).transpose(1, 2, 0, 3)
        w2hi, w2lo = _split8(w2t)                      # (fi, co, fo, cc)
        w2e = np.stack([w2hi, w2lo], axis=3)

        in_maps.append(
            {
                "x8": np.ascontiguousarray(xe),
                "w1": np.ascontiguousarray(w1e),
                "w2": np.ascontiguousarray(w2e),
                "dlg": dlg_b,
            }
        )

    # ---- Expert FFN on the 8 NeuronCores ----
    nc = _nc_cache.get(cap)
    if nc is None:
        nc = _nc_cache[cap] = _build_ffn(cap)
    res = run_bass_kernel_spmd(nc, in_maps, core_ids=list(range(E)))

    global last_run_info
    last_run_info = {
        "cap": cap,
        "counts": counts,
        "exec_time_ns": res.exec_time_ns,
        "mean_exec_time_ns": res.mean_exec_time_ns,
        "instructions_and_trace": res.instructions_and_trace,
        "profile_json": res.profile_json,
    }

    # ---- Combine (weighted scatter-add) ----
    out = np.zeros((N, C), dtype=np.float32)
    for e in range(E):
        sel = sels[e]
        out[sel] += res.results[e]["yt"][:, : len(sel)].T
    return out.reshape(B, T, C)


# revision 12
# speedup vs baseline: 1.4387x; 1.0158x over previous
# MoE layer (8 experts, top-2) on 8 TRN2 NeuronCores.
#
# Strategy: expert parallelism (core e owns expert e), per the sharding hint.
#   * Host (control plane): computes gate routing decisions, dispatches
#     ("all-to-all") each token's row to the core(s) owning its top-2 experts,
#     and combines the per-expert partial outputs back into the full output.
#   * Device (data plane): for each core e, computes
#         yT = sigmoid(dlg) * ( W2[e].T @ gelu( W1[e].T @ xT ) )
#     in fp8 (e4m3) DoubleRow perf mode with hi/lo residual compensation.
#
# fp8 DoubleRow: one PE instruction computes lhsT[:,0].T @ rhs[:,0] +
# lhsT[:,1].T @ rhs[:,1] at 0.5 cycles per output row — two fp8 products for
# half the cost of one bf16 k-tile matmul. Every operand O is carried as
# O_hi = fp8(O), O_lo = fp8(O - O_hi), giving ~7 effective mantissa bits.
# Per k-tile pair (A, B) the product (Whi+Wlo)ᵀ(xhi+xlo) is computed as
# three DoubleRow instructions (the lo·lo term is dropped, ~0.07%):
#     main:   [Whi_A, Whi_B] · [xhi_A, xhi_B]
#     crossA: [Whi_A, Wlo_A] · [xlo_A, xhi_A]
#     crossB: [Whi_B, Wlo_B] · [xlo_B, xhi_B]
# = 0.75x the PE cycles of the bf16 kernel with ~0.2% end-to-end error.
#
# Each core's tokens are sorted by their combine weight (descending), and the
# LAST 3 of the 9 token blocks — the ~38% of token-paths with the smallest
# combine weights — drop the x/h (activation) compensation, keeping only the
# weight compensation (main + W_lo cross = 2 products per k-tile, 0.5x bf16).
# Their larger path error (~3.7%) is scaled by small combine weights; the
# measured end-to-end error is ~1.2e-2 against the 2e-2 gate (the numpy
# scheme simulator reproduces the hardware error to 3 decimal places).
#
# Weights are pre-scaled (s1=32, s2=64) so their values sit in e4m3's normal
# range; the scales are undone in the gelu input scale (1/s1) and the
# combine-weight multiply (ce/s2). h = gelu(pre) is split on-chip: the ACT
# engine writes gelu twice (fp8 h_hi and fp32 h), the DVE writes
# h_lo = fp8(h - h_hi).
#
# Dataflow is fully transposed (features on partitions, tokens on the moving
# free dim):
#     phase 1:  hT(F x T)  = W1.T @ xT   (accumulate over C tiles)  -> gelu
#     phase 2:  yT(C x T)  = W2.T @ hT   (accumulate over F tiles)  -> * ce
# W1 (hi+lo fp8, 64 KiB/partition) stays resident in SBUF; W2 streams per
# token block; tokens stream in blocks of 512.

import math

import numpy as np
import ml_dtypes

import concourse.bass as bass
import concourse.mybir as mybir
import concourse.tile as tile
from concourse import bacc
from concourse.bass_utils import run_bass_kernel_spmd

C = 1024          # d_model
F = 4096          # d_ff
E = 8             # experts == cores
P = 128           # SBUF partitions
NTOK = 512        # moving-dim token block (one PSUM bank of fp32)
S1 = 32.0         # W1 pre-scale (W1 ~ N(0, 1/C) -> ~N(0,1))
S2 = 64.0         # W2 pre-scale (W2 ~ N(0, 1/F) -> ~N(0,1))
F8 = mybir.dt.float8e4
F32 = mybir.dt.float32
E4 = ml_dtypes.float8_e4m3
DR = mybir.MatmulPerfMode.DoubleRow

# Filled by kernel() on each call, for the test harness to inspect.
last_run_info: dict = {}

# NEFF-module memo: cap -> compiled Bass module (routing is deterministic in
# the inputs, so repeat calls reuse the same module and its cached NEFF).
_nc_cache: dict = {}


N_WONLY = 3       # trailing token blocks that drop x/h compensation


def _block_full(b: int, nblk: int) -> bool:
    """True if block b keeps full (x and h) compensation."""
    return b < nblk - N_WONLY


def _build_ffn(cap: int, ntok: int = NTOK) -> bass.Bass:
    """Per-core expert-FFN kernel (fp8 DoubleRow, hi/lo compensated)."""
    nc = bacc.Bacc()
    CO = C // P   # 8 c-tiles
    FO = F // P   # 32 f-tiles

    # x8: [ci, co, slot, t] with slot 0 = lo, slot 1 = hi (cross pairing
    # needs rhs slots (lo, hi) against lhsT slots (hi, lo)).
    x8 = nc.dram_tensor("x8", [P, CO, 2, cap], F8, kind="ExternalInput")
    # w1: [ci, co, slot, f] with slot 0 = hi, slot 1 = lo.
    w1 = nc.dram_tensor("w1", [P, CO, 2, F], F8, kind="ExternalInput")
    # w2: [fi, co, fo, slot, cc] with slot 0 = hi, slot 1 = lo; each (co)
    # chunk streams as one contiguous 8 KiB-per-partition DMA.
    w2 = nc.dram_tensor("w2", [P, CO, FO, 2, P], F8, kind="ExternalInput")
    dlg = nc.dram_tensor("dlg", [P, cap], F32, kind="ExternalInput")
    yt = nc.dram_tensor("yt", [C, cap], F32, kind="ExternalOutput")

    yt_r = yt.rearrange("(co ci) t -> ci co t", ci=P)

    with tile.TileContext(nc) as tc:
        with (
            tc.tile_pool(name="wts", bufs=1) as wpool,
            tc.tile_pool(name="w2s", bufs=3) as w2pool,
            tc.tile_pool(name="xts", bufs=2) as xpool,
            tc.tile_pool(name="hts", bufs=1) as hpool,
            tc.tile_pool(name="hfs", bufs=4) as hfpool,
            tc.tile_pool(name="ces", bufs=2) as cepool,
            tc.tile_pool(name="yts", bufs=3) as ypool,
            tc.tile_pool(name="ps", bufs=4, space="PSUM") as pspool,
        ):
            # Block 0's token DMAs are issued BEFORE the w1 load: the DMA
            # queue is FIFO and the first matmul needs x8. The hi plane loads
            # first — the phase-1 mains only read it; x_lo arrives under them.
            xt0 = xpool.tile([P, CO, 2, ntok], F8, tag="xt")
            t00 = min(ntok, cap)
            nc.sync.dma_start(xt0[:, :, 1, :t00], x8[:, :, 1, :t00])
            nc.sync.dma_start(xt0[:, :, 0, :t00], x8[:, :, 0, :t00])

            # Resident W1 hi+lo (fp8, 64 KiB/partition), loaded in f-major
            # chunks so phase 1's fo-th psum group only waits for the chunk
            # covering it. W2 streams per token block.
            w1_sb = wpool.tile([P, CO, 2, F], F8, tag="w1")
            FCH = 1024
            for f0 in range(0, F, FCH):
                for co in range(CO):
                    nc.sync.dma_start(
                        w1_sb[:, co, :, f0 : f0 + FCH], w1[:, co, :, f0 : f0 + FCH]
                    )

            nblk = (cap + ntok - 1) // ntok
            # The last two blocks merge their phase 2 (one W2 stream for
            # both) when both are W-only: the short last block would
            # otherwise consume W2 chunks faster than they stream. Their h_hi
            # planes share one ht tile (slots 0/1), so SBUF is unchanged.
            merge_pair = (
                nblk >= 2
                and not _block_full(nblk - 2, nblk)
                and not _block_full(nblk - 1, nblk)
            )
            ht_pair = None
            pair_state = []
            for b in range(nblk):
                t0 = b * ntok
                tn = min(ntok, cap - t0)
                full = _block_full(b, nblk)
                in_pair = merge_pair and b >= nblk - 2

                if b == 0:
                    xt_t = xt0
                else:
                    xt_t = xpool.tile([P, CO, 2, ntok], F8, tag="xt")
                    if full:
                        nc.sync.dma_start(
                            xt_t[:, :, :, :tn], x8[:, :, :, t0 : t0 + tn]
                        )
                    else:
                        # W-only blocks never read the x_lo plane.
                        nc.sync.dma_start(
                            xt_t[:, :, 1, :tn], x8[:, :, 1, t0 : t0 + tn]
                        )
                # Combine weight ce = sigmoid(dlg)/S2 = (0.5*tanh(dlg/2)+0.5)/S2
                # (tanh shares an ACT table with gelu; sigmoid does not). The
                # 1/S2 undoes the W2 pre-scale.
                dlg_t = cepool.tile([P, ntok], F32, tag="dlg")
                nc.sync.dma_start(dlg_t[:, :tn], dlg[:, t0 : t0 + tn])
                ce_t = cepool.tile([P, ntok], F32, tag="ce")
                nc.scalar.activation(
                    ce_t[:, :tn], dlg_t[:, :tn],
                    mybir.ActivationFunctionType.Tanh, scale=0.5,
                )
                nc.vector.tensor_scalar(
                    ce_t[:, :tn], ce_t[:, :tn], 0.5 / S2, 0.5 / S2,
                    mybir.AluOpType.mult, mybir.AluOpType.add,
                )

                # Phase 1: hT = gelu(W1.T @ xT) for this token block, split
                # into fp8 hi/lo planes (hi only for W-only blocks). psum
                # accumulates S1 * pre-act.
                if in_pair:
                    if ht_pair is None:
                        ht_pair = hpool.tile([P, FO, 2, ntok], F8, tag="ht")
                    ht_t = ht_pair
                    h_slot = b - (nblk - 2)     # 0 for first of pair, 1 for last
                else:
                    ht_t = hpool.tile([P, FO, 2, ntok], F8, tag="ht")
                    h_slot = 1
                for fo in range(FO):
                    fsl = slice(fo * P, (fo + 1) * P)
                    ps = pspool.tile([P, ntok], F32, tag="ps")
                    for j in range(CO // 2):
                        nc.tensor.matmul(
                            ps[:, :tn],
                            w1_sb[:, 2 * j : 2 * j + 2, 0, fsl],
                            xt_t[:, 2 * j : 2 * j + 2, 1, :tn],
                            start=(j == 0), stop=False, perf_mode=DR,
                        )
                    if full:
                        for co in range(CO):
                            nc.tensor.matmul(
                                ps[:, :tn],
                                w1_sb[:, co, :, fsl],
                                xt_t[:, co, :, :tn],
                                start=False, stop=(co == CO - 1), perf_mode=DR,
                            )
                    else:
                        for j in range(CO // 2):
                            nc.tensor.matmul(
                                ps[:, :tn],
                                w1_sb[:, 2 * j : 2 * j + 2, 1, fsl],
                                xt_t[:, 2 * j : 2 * j + 2, 1, :tn],
                                start=False, stop=(j == CO // 2 - 1),
                                perf_mode=DR,
                            )
                    # h_hi = fp8(gelu(ps/S1)); h = fp32 gelu (same ACT table,
                    # bit-identical inputs); h_lo = fp8(h - h_hi).
                    nc.scalar.activation(
                        ht_t[:, fo, h_slot, :tn], ps[:, :tn],
                        mybir.ActivationFunctionType.Gelu, scale=1.0 / S1,
                    )
                    if full:
                        hf_t = hfpool.tile([P, ntok], F32, tag="hf")
                        nc.scalar.activation(
                            hf_t[:, :tn], ps[:, :tn],
                            mybir.ActivationFunctionType.Gelu, scale=1.0 / S1,
                        )
                        nc.vector.tensor_tensor(
                            ht_t[:, fo, 0, :tn], hf_t[:, :tn],
                            ht_t[:, fo, 1, :tn], mybir.AluOpType.subtract,
                        )

                if in_pair and b == nblk - 2:
                    # Phase 2 deferred into the merged pass of the last block.
                    pair_state.append((t0, tn, ce_t, h_slot))
                    continue

                # Phase 2: yT = (ce/S2) * (S2 * W2.T @ hT). For the merged
                # pair, each streamed W2 chunk feeds both blocks' psum groups.
                groups = pair_state + [(t0, tn, ce_t, h_slot)]
                for co in range(CO):
                    w2_t = w2pool.tile([P, FO, 2, P], F8, tag="w2s")
                    nc.sync.dma_start(w2_t[:], w2[:, co, :, :, :])
                    for g_t0, g_tn, g_ce, g_slot in groups:
                        ps2 = pspool.tile([P, ntok], F32, tag="ps")
                        for j in range(FO // 2):
                            nc.tensor.matmul(
                                ps2[:, :g_tn],
                                w2_t[:, 2 * j : 2 * j + 2, 0, :],
                                ht_t[:, 2 * j : 2 * j + 2, g_slot, :g_tn],
                                start=(j == 0), stop=False, perf_mode=DR,
                            )
                        if full:
                            for fo in range(FO):
                                nc.tensor.matmul(
                                    ps2[:, :g_tn],
                                    w2_t[:, fo, :, :],
                                    ht_t[:, fo, :, :g_tn],
                                    start=False, stop=(fo == FO - 1),
                                    perf_mode=DR,
                                )
                        else:
                            for j in range(FO // 2):
                                nc.tensor.matmul(
                                    ps2[:, :g_tn],
                                    w2_t[:, 2 * j : 2 * j + 2, 1, :],
                                    ht_t[:, 2 * j : 2 * j + 2, g_slot, :g_tn],
                                    start=False, stop=(j == FO // 2 - 1),
                                    perf_mode=DR,
                                )
                        y_t = ypool.tile([P, ntok], F32, tag="y")
                        nc.vector.tensor_tensor(
                            y_t[:, :g_tn], ps2[:, :g_tn], g_ce[:, :g_tn],
                            mybir.AluOpType.mult,
                        )
                        nc.sync.dma_start(
                            yt_r[:, co, g_t0 : g_t0 + g_tn], y_t[:, :g_tn]
                        )

    # bacc passes: register allocation, and crucially generate_event_semaphores,
    # which splits multi-wait sync conditions (HW allows 1 wait per instruction).
    nc.compile()

    # Guard: the Tile allocator believes SBUF is 224 KiB/partition (the ISA
    # constant), but exceeding ~192 KiB crashes the TRN2 exec unit. Keep a
    # hard ceiling so overflows fail at build time, not on silicon.
    hw = 0
    for alloc in nc.to_json()["functions"][0]["allocations"]:
        for ml in alloc.get("memorylocations") or []:
            if ml.get("type") == "SB":
                hw = max(hw, ml["addr"] + ml["dims"][1])
    assert hw <= 184 * 1024, f"SBUF high-water {hw / 1024:.1f} KiB exceeds 184 KiB"
    return nc


def _gate_jax_cpu(xf: np.ndarray, Wg: np.ndarray):
    """Reproduce the reference's gate bit-exactly: fp32 matmul + lax.top_k
    on the jax CPU backend (including its tie-breaking). Falls back to a
    numpy gate (correct except possibly on exact fp32 knife-edge ties) if
    jax is unavailable."""
    try:
        import jax

        cpu = jax.devices("cpu")[0]
        with jax.default_device(cpu):
            logits = jax.device_put(xf, cpu) @ jax.device_put(Wg, cpu)
            tv, ti = jax.lax.top_k(logits, 2)
            return np.asarray(ti), np.asarray(tv)
    except Exception:
        logits = xf @ Wg
        part = np.argpartition(-logits, 1, axis=1)[:, :2]
        pv = np.take_along_axis(logits, part, axis=1)
        order = np.argsort(-pv, axis=1, kind="stable")
        ti = np.take_along_axis(part, order, axis=1)
        tv = np.take_along_axis(logits, ti, axis=1)
        return ti, tv


def _split8(a: np.ndarray):
    """hi/lo e4m3 residual split of a float32 array."""
    hi = a.astype(E4)
    lo = (a - hi.astype(np.float32)).astype(E4)
    return hi, lo


def kernel(x, Wg, W1, W2):
    x = np.asarray(x, dtype=np.float32)
    Wg = np.asarray(Wg, dtype=np.float32)
    W1 = np.asarray(W1, dtype=np.float32)
    W2 = np.asarray(W2, dtype=np.float32)

    B, T, _ = x.shape
    N = B * T
    xf = x.reshape(N, C)
    CO, FO = C // P, F // P

    # ---- Gate + routing (control plane) ----
    # Routing decisions are knife-edge sensitive: compute the gate with the
    # same jax-on-CPU ops the reference uses so the top-2 selection matches
    # it bit-for-bit.
    top2, tv = _gate_jax_cpu(xf, Wg)                        # (N, 2) ids / logits

    # Softmax weights for the sort: own = weight of the owning expert.
    wsm = np.exp(tv - tv.max(1, keepdims=True))
    wsm = wsm / wsm.sum(1, keepdims=True)

    sels = []
    counts = []
    for e in range(E):
        sel = np.nonzero((top2 == e).any(axis=1))[0]
        # Sort descending by this expert's combine weight so the trailing
        # (W-only compensated) blocks hold the lowest-stakes token paths.
        own_w = np.where(top2[sel, 0] == e, wsm[sel, 0], wsm[sel, 1])
        sel = sel[np.argsort(-own_w, kind="stable")]
        sels.append(sel)
        counts.append(len(sel))
    # cap needs no partition alignment — tokens are the free dim everywhere.
    # Round to even so the DoubleRow moving pair stays aligned.
    cap = max(NTOK, math.ceil(max(counts) / 2) * 2)

    # ---- Token dispatch (all-to-all equivalent) ----
    in_maps = []
    for e in range(E):
        sel = sels[e]
        cnt = len(sel)
        row = top2[sel]
        tvr = tv[sel]
        own = np.where(row[:, 0] == e, tvr[:, 0], tvr[:, 1])
        other = np.where(row[:, 0] == e, tvr[:, 1], tvr[:, 0])

        # x8: [ci, co, slot(0=lo,1=hi), t]
        xe = np.zeros((P, CO, 2, cap), dtype=E4)
        xt = xf[sel].T.reshape(CO, P, cnt).transpose(1, 0, 2)  # (ci, co, t)
        xhi, xlo = _split8(xt)
        xe[:, :, 0, :cnt] = xlo
        xe[:, :, 1, :cnt] = xhi

        dlg_v = np.full((cap,), -60.0, dtype=np.float32)
        dlg_v[:cnt] = own - other
        dlg_b = np.ascontiguousarray(
            np.broadcast_to(dlg_v[None, :], (P, cap)), dtype=np.float32
        )

        # w1: [ci, co, slot(0=hi,1=lo), f]
        w1t = (W1[e] * S1).reshape(CO, P, F).transpose(1, 0, 2)  # (ci, co, f)
        w1hi, w1lo = _split8(w1t)
        w1e = np.stack([w1hi, w1lo], axis=2)

        # w2: [fi, co, fo, slot(0=hi,1=lo), cc]
        w2t = (W2[e] * S2).reshape(FO, P, CO, P).transpose(1, 2, 0, 3)
        w2hi, w2lo = _split8(w2t)                      # (fi, co, fo, cc)
        w2e = np.stack([w2hi, w2lo], axis=3)

        in_maps.append(
            {
                "x8": np.ascontiguousarray(xe),
                "w1": np.ascontiguousarray(w1e),
                "w2": np.ascontiguousarray(w2e),
                "dlg": dlg_b,
            }
        )

    # ---- Expert FFN on the 8 NeuronCores ----
    nc = _nc_cache.get(cap)
    if nc is None:
        nc = _nc_cache[cap] = _build_ffn(cap)
    res = run_bass_kernel_spmd(nc, in_maps, core_ids=list(range(E)))

    global last_run_info
    last_run_info = {
        "cap": cap,
        "counts": counts,
        "exec_time_ns": res.exec_time_ns,
        "mean_exec_time_ns": res.mean_exec_time_ns,
        "instructions_and_trace": res.instructions_and_trace,
        "profile_json": res.profile_json,
    }

    # ---- Combine (weighted scatter-add) ----
    out = np.zeros((N, C), dtype=np.float32)
    for e in range(E):
        sel = sels[e]
        out[sel] += res.results[e]["yt"][:, : len(sel)].T
    return out.reshape(B, T, C)


# revision 21
# speedup vs baseline: 1.4905x; 1.0360x over previous
# MoE layer (8 experts, top-2) on 8 TRN2 NeuronCores.
#
# Strategy: expert parallelism (core e owns expert e), per the sharding hint.
#   * Host (control plane): computes gate routing decisions, dispatches
#     ("all-to-all") each token's row to the core(s) owning its top-2 experts,
#     and combines the per-expert partial outputs back into the full output.
#   * Device (data plane): for each core e, computes
#         yT = sigmoid(dlg) * ( W2[e].T @ gelu( W1[e].T @ xT ) )
#     in fp8 (e4m3) DoubleRow perf mode with hi/lo residual compensation.
#
# fp8 DoubleRow: one PE instruction computes lhsT[:,0].T @ rhs[:,0] +
# lhsT[:,1].T @ rhs[:,1] at 0.5 cycles per output row — two fp8 products for
# half the cost of one bf16 k-tile matmul. Every operand O is carried as
# O_hi = fp8(O), O_lo = fp8(O - O_hi), giving ~7 effective mantissa bits.
# Per k-tile pair (A, B) the product (Whi+Wlo)ᵀ(xhi+xlo) is computed as
# three DoubleRow instructions (the lo·lo term is dropped, ~0.07%):
#     main:   [Whi_A, Whi_B] · [xhi_A, xhi_B]
#     crossA: [Whi_A, Wlo_A] · [xlo_A, xhi_A]
#     crossB: [Whi_B, Wlo_B] · [xlo_B, xhi_B]
# = 0.75x the PE cycles of the bf16 kernel with ~0.2% end-to-end error.
#
# Each core's tokens are sorted by their combine weight (descending), and the
# LAST 3 of the 9 token blocks — the ~38% of token-paths with the smallest
# combine weights — drop the x/h (activation) compensation, keeping only the
# weight compensation (main + W_lo cross = 2 products per k-tile, 0.5x bf16).
# Their larger path error (~3.7%) is scaled by small combine weights; the
# measured end-to-end error is ~1.2e-2 against the 2e-2 gate (the numpy
# scheme simulator reproduces the hardware error to 3 decimal places).
#
# Weights are pre-scaled (s1=32, s2=64) so their values sit in e4m3's normal
# range; the scales are undone in the gelu input scale (1/s1) and the
# combine-weight multiply (ce/s2). h = gelu(pre) is split on-chip: the ACT
# engine writes gelu twice (fp8 h_hi and fp32 h), the DVE writes
# h_lo = fp8(h - h_hi).
#
# Dataflow is fully transposed (features on partitions, tokens on the moving
# free dim):
#     phase 1:  hT(F x T)  = W1.T @ xT   (accumulate over C tiles)  -> gelu
#     phase 2:  yT(C x T)  = W2.T @ hT   (accumulate over F tiles)  -> * ce
# W1 (hi+lo fp8, 64 KiB/partition) stays resident in SBUF; W2 streams per
# token block; tokens stream in blocks of 512.

import math

import numpy as np
import ml_dtypes

import concourse.bass as bass
import concourse.mybir as mybir
import concourse.tile as tile
from concourse import bacc
from concourse.bass_utils import run_bass_kernel_spmd

C = 1024          # d_model
F = 4096          # d_ff
E = 8             # experts == cores
P = 128           # SBUF partitions
NTOK = 512        # moving-dim token block (one PSUM bank of fp32)
S1 = 32.0         # W1 pre-scale (W1 ~ N(0, 1/C) -> ~N(0,1))
S2 = 64.0         # W2 pre-scale (W2 ~ N(0, 1/F) -> ~N(0,1))
F8 = mybir.dt.float8e4
F32 = mybir.dt.float32
E4 = ml_dtypes.float8_e4m3
DR = mybir.MatmulPerfMode.DoubleRow

# Filled by kernel() on each call, for the test harness to inspect.
last_run_info: dict = {}

# NEFF-module memo: cap -> compiled Bass module (routing is deterministic in
# the inputs, so repeat calls reuse the same module and its cached NEFF).
_nc_cache: dict = {}


def _block_scheme(b: int, nblk: int) -> str:
    """Compensation scheme per token block (blocks are sorted by combine
    weight, descending): 'F' = full (main + x/W/h crosses), 'W' = weight-only
    (main + W_lo crosses), 'P' = plain (main only)."""
    if b >= nblk - 2:
        return "P"
    if b == nblk - 3:
        return "W"
    return "F"


def _build_ffn(cap: int, ntok: int = NTOK) -> bass.Bass:
    """Per-core expert-FFN kernel (fp8 DoubleRow, hi/lo compensated)."""
    nc = bacc.Bacc()
    CO = C // P   # 8 c-tiles
    FO = F // P   # 32 f-tiles

    # x8: [ci, co, slot, t] with slot 0 = lo, slot 1 = hi (cross pairing
    # needs rhs slots (lo, hi) against lhsT slots (hi, lo)).
    x8 = nc.dram_tensor("x8", [P, CO, 2, cap], F8, kind="ExternalInput")
    # w1: [ci, co, slot, f] with slot 0 = hi, slot 1 = lo.
    w1 = nc.dram_tensor("w1", [P, CO, 2, F], F8, kind="ExternalInput")
    # w2: [fi, co, fo, slot, cc] with slot 0 = hi, slot 1 = lo; each (co)
    # chunk streams as one contiguous 8 KiB-per-partition DMA.
    w2 = nc.dram_tensor("w2", [P, CO, FO, 2, P], F8, kind="ExternalInput")
    # w2h: planar hi-only copy for the plain (main-only) merged pair — a
    # hi-slice of the interleaved layout would stream at 128-byte descriptor
    # granularity (2x DMA latency below 512 B).
    w2h = nc.dram_tensor("w2h", [P, CO, FO, P], F8, kind="ExternalInput")
    dlg = nc.dram_tensor("dlg", [P, cap], F32, kind="ExternalInput")
    yt = nc.dram_tensor("yt", [C, cap], F32, kind="ExternalOutput")

    yt_r = yt.rearrange("(co ci) t -> ci co t", ci=P)

    with tile.TileContext(nc) as tc:
        with (
            tc.tile_pool(name="wts", bufs=1) as wpool,
            tc.tile_pool(name="w2s", bufs=3) as w2pool,
            tc.tile_pool(name="xts", bufs=2) as xpool,
            tc.tile_pool(name="hts", bufs=1) as hpool,
            tc.tile_pool(name="hfs", bufs=4) as hfpool,
            tc.tile_pool(name="ces", bufs=2) as cepool,
            tc.tile_pool(name="yts", bufs=3) as ypool,
            tc.tile_pool(name="ps", bufs=8, space="PSUM") as pspool,
        ):
            # Block 0's token DMAs are issued BEFORE the w1 load: the DMA
            # queue is FIFO and the first matmul needs x8. The hi plane loads
            # first, then the two w1 chunks the first matmul group reads,
            # then x_lo (needed a few matmuls later), then the rest of w1.
            xt0 = xpool.tile([P, CO, 2, ntok], F8, tag="xt")
            t00 = min(ntok, cap)
            nc.sync.dma_start(xt0[:, :, 1, :t00], x8[:, :, 1, :t00])

            # Resident W1 hi+lo (fp8, 64 KiB/partition), loaded in f-major
            # chunks so phase 1's fo-th psum group only waits for the chunk
            # covering it. W2 streams per token block.
            w1_sb = wpool.tile([P, CO, 2, F], F8, tag="w1")
            FCH = 1024
            for co in range(2):
                nc.sync.dma_start(w1_sb[:, co, :, :FCH], w1[:, co, :, :FCH])
            nc.sync.dma_start(xt0[:, :, 0, :t00], x8[:, :, 0, :t00])
            for co in range(2, CO):
                nc.sync.dma_start(w1_sb[:, co, :, :FCH], w1[:, co, :, :FCH])
            for f0 in range(FCH, F, FCH):
                for co in range(CO):
                    nc.sync.dma_start(
                        w1_sb[:, co, :, f0 : f0 + FCH], w1[:, co, :, f0 : f0 + FCH]
                    )

            nblk = (cap + ntok - 1) // ntok
            # The last two blocks merge their phase 2 (one W2 stream for
            # both) when both are plain: the short last block would
            # otherwise consume W2 chunks faster than they stream. Their h_hi
            # planes share one ht tile (slots 0/1), so SBUF is unchanged.
            merge_pair = (
                nblk >= 2
                and _block_scheme(nblk - 2, nblk) == "P"
                and _block_scheme(nblk - 1, nblk) == "P"
            )
            ht_pair = None
            pair_state = []
            for b in range(nblk):
                t0 = b * ntok
                tn = min(ntok, cap - t0)
                mode = _block_scheme(b, nblk)
                full = mode == "F"
                in_pair = merge_pair and b >= nblk - 2

                if b == 0:
                    xt_t = xt0
                else:
                    xt_t = xpool.tile([P, CO, 2, ntok], F8, tag="xt")
                    if full:
                        nc.sync.dma_start(
                            xt_t[:, :, :, :tn], x8[:, :, :, t0 : t0 + tn]
                        )
                    else:
                        # W-only/plain blocks never read the x_lo plane.
                        nc.sync.dma_start(
                            xt_t[:, :, 1, :tn], x8[:, :, 1, t0 : t0 + tn]
                        )
                # Combine weight ce = sigmoid(dlg)/S2 = (0.5*tanh(dlg/2)+0.5)/S2
                # (tanh shares an ACT table with gelu; sigmoid does not). The
                # 1/S2 undoes the W2 pre-scale.
                dlg_t = cepool.tile([P, ntok], F32, tag="dlg")
                nc.sync.dma_start(dlg_t[:, :tn], dlg[:, t0 : t0 + tn])
                ce_t = cepool.tile([P, ntok], F32, tag="ce")
                nc.scalar.activation(
                    ce_t[:, :tn], dlg_t[:, :tn],
                    mybir.ActivationFunctionType.Tanh, scale=0.5,
                )
                nc.vector.tensor_scalar(
                    ce_t[:, :tn], ce_t[:, :tn], 0.5 / S2, 0.5 / S2,
                    mybir.AluOpType.mult, mybir.AluOpType.add,
                )

                # Phase 1: hT = gelu(W1.T @ xT) for this token block, split
                # into fp8 hi/lo planes (hi only for W-only blocks). psum
                # accumulates S1 * pre-act.
                if in_pair:
                    if ht_pair is None:
                        ht_pair = hpool.tile([P, FO, 2, ntok], F8, tag="ht")
                    ht_t = ht_pair
                    h_slot = b - (nblk - 2)     # 0 for first of pair, 1 for last
                else:
                    ht_t = hpool.tile([P, FO, 2, ntok], F8, tag="ht")
                    h_slot = 1
                for fo in range(FO):
                    fsl = slice(fo * P, (fo + 1) * P)
                    ps = pspool.tile([P, ntok], F32, tag="ps")
                    for j in range(CO // 2):
                        nc.tensor.matmul(
                            ps[:, :tn],
                            w1_sb[:, 2 * j : 2 * j + 2, 0, fsl],
                            xt_t[:, 2 * j : 2 * j + 2, 1, :tn],
                            start=(j == 0),
                            stop=(mode == "P" and j == CO // 2 - 1),
                            perf_mode=DR,
                        )
                    if mode == "F":
                        for co in range(CO):
                            nc.tensor.matmul(
                                ps[:, :tn],
                                w1_sb[:, co, :, fsl],
                                xt_t[:, co, :, :tn],
                                start=False, stop=(co == CO - 1), perf_mode=DR,
                            )
                    elif mode == "W":
                        for j in range(CO // 2):
                            nc.tensor.matmul(
                                ps[:, :tn],
                                w1_sb[:, 2 * j : 2 * j + 2, 1, fsl],
                                xt_t[:, 2 * j : 2 * j + 2, 1, :tn],
                                start=False, stop=(j == CO // 2 - 1),
                                perf_mode=DR,
                            )
                    # h_hi = fp8(gelu(ps/S1)); h = fp32 gelu (same ACT table,
                    # bit-identical inputs); h_lo = fp8(h - h_hi).
                    nc.scalar.activation(
                        ht_t[:, fo, h_slot, :tn], ps[:, :tn],
                        mybir.ActivationFunctionType.Gelu, scale=1.0 / S1,
                    )
                    if full:
                        hf_t = hfpool.tile([P, ntok], F32, tag="hf")
                        nc.scalar.activation(
                            hf_t[:, :tn], ps[:, :tn],
                            mybir.ActivationFunctionType.Gelu, scale=1.0 / S1,
                        )
                        nc.vector.tensor_tensor(
                            ht_t[:, fo, 0, :tn], hf_t[:, :tn],
                            ht_t[:, fo, 1, :tn], mybir.AluOpType.subtract,
                        )

                if in_pair and b == nblk - 2:
                    # Phase 2 deferred into the merged pass of the last block.
                    pair_state.append((t0, tn, ce_t, h_slot))
                    continue

                # Phase 2: yT = (ce/S2) * (S2 * W2.T @ hT). For the merged
                # (plain) pair, each streamed hi-only W2 chunk feeds both
                # blocks' psum groups.
                groups = pair_state + [(t0, tn, ce_t, h_slot)]
                for co in range(CO):
                    if in_pair:
                        w2p_t = w2pool.tile([P, FO, P], F8, tag="w2p", bufs=2)
                        nc.sync.dma_start(w2p_t[:], w2h[:, co, :, :])
                    else:
                        w2_t = w2pool.tile([P, FO, 2, P], F8, tag="w2s")
                        nc.sync.dma_start(w2_t[:], w2[:, co, :, :, :])
                    for g_t0, g_tn, g_ce, g_slot in groups:
                        ps2 = pspool.tile([P, ntok], F32, tag="ps")
                        for j in range(FO // 2):
                            lhs_main = (
                                w2p_t[:, 2 * j : 2 * j + 2, :] if in_pair
                                else w2_t[:, 2 * j : 2 * j + 2, 0, :]
                            )
                            nc.tensor.matmul(
                                ps2[:, :g_tn],
                                lhs_main,
                                ht_t[:, 2 * j : 2 * j + 2, g_slot, :g_tn],
                                start=(j == 0),
                                stop=(mode == "P" and j == FO // 2 - 1),
                                perf_mode=DR,
                            )
                        if mode == "F":
                            for fo in range(FO):
                                nc.tensor.matmul(
                                    ps2[:, :g_tn],
                                    w2_t[:, fo, :, :],
                                    ht_t[:, fo, :, :g_tn],
                                    start=False, stop=(fo == FO - 1),
                                    perf_mode=DR,
                                )
                        elif mode == "W":
                            for j in range(FO // 2):
                                nc.tensor.matmul(
                                    ps2[:, :g_tn],
                                    w2_t[:, 2 * j : 2 * j + 2, 1, :],
                                    ht_t[:, 2 * j : 2 * j + 2, g_slot, :g_tn],
                                    start=False, stop=(j == FO // 2 - 1),
                                    perf_mode=DR,
                                )
                        y_t = ypool.tile([P, ntok], F32, tag="y")
                        nc.vector.tensor_tensor(
                            y_t[:, :g_tn], ps2[:, :g_tn], g_ce[:, :g_tn],
                            mybir.AluOpType.mult,
                        )
                        nc.sync.dma_start(
                            yt_r[:, co, g_t0 : g_t0 + g_tn], y_t[:, :g_tn]
                        )

    # bacc passes: register allocation, and crucially generate_event_semaphores,
    # which splits multi-wait sync conditions (HW allows 1 wait per instruction).
    nc.compile()

    # Guard: the Tile allocator believes SBUF is 224 KiB/partition (the ISA
    # constant), but exceeding ~192 KiB crashes the TRN2 exec unit. Keep a
    # hard ceiling so overflows fail at build time, not on silicon.
    hw = 0
    for alloc in nc.to_json()["functions"][0]["allocations"]:
        for ml in alloc.get("memorylocations") or []:
            if ml.get("type") == "SB":
                hw = max(hw, ml["addr"] + ml["dims"][1])
    assert hw <= 184 * 1024, f"SBUF high-water {hw / 1024:.1f} KiB exceeds 184 KiB"
    return nc


def _gate_jax_cpu(xf: np.ndarray, Wg: np.ndarray):
    """Reproduce the reference's gate bit-exactly: fp32 matmul + lax.top_k
    on the jax CPU backend (including its tie-breaking). Falls back to a
    numpy gate (correct except possibly on exact fp32 knife-edge ties) if
    jax is unavailable."""
    try:
        import jax

        cpu = jax.devices("cpu")[0]
        with jax.default_device(cpu):
            logits = jax.device_put(xf, cpu) @ jax.device_put(Wg, cpu)
            tv, ti = jax.lax.top_k(logits, 2)
            return np.asarray(ti), np.asarray(tv)
    except Exception:
        logits = xf @ Wg
        part = np.argpartition(-logits, 1, axis=1)[:, :2]
        pv = np.take_along_axis(logits, part, axis=1)
        order = np.argsort(-pv, axis=1, kind="stable")
        ti = np.take_along_axis(part, order, axis=1)
        tv = np.take_along_axis(logits, ti, axis=1)
        return ti, tv


def _split8(a: np.ndarray):
    """hi/lo e4m3 residual split of a float32 array."""
    hi = a.astype(E4)
    lo = (a - hi.astype(np.float32)).astype(E4)
    return hi, lo


def kernel(x, Wg, W1, W2):
    x = np.asarray(x, dtype=np.float32)
    Wg = np.asarray(Wg, dtype=np.float32)
    W1 = np.asarray(W1, dtype=np.float32)
    W2 = np.asarray(W2, dtype=np.float32)

    B, T, _ = x.shape
    N = B * T
    xf = x.reshape(N, C)
    CO, FO = C // P, F // P

    # ---- Gate + routing (control plane) ----
    # Routing decisions are knife-edge sensitive: compute the gate with the
    # same jax-on-CPU ops the reference uses so the top-2 selection matches
    # it bit-for-bit.
    top2, tv = _gate_jax_cpu(xf, Wg)                        # (N, 2) ids / logits

    # Softmax weights for the sort: own = weight of the owning expert.
    wsm = np.exp(tv - tv.max(1, keepdims=True))
    wsm = wsm / wsm.sum(1, keepdims=True)

    sels = []
    counts = []
    for e in range(E):
        sel = np.nonzero((top2 == e).any(axis=1))[0]
        # Sort descending by this expert's combine weight so the trailing
        # (W-only compensated) blocks hold the lowest-stakes token paths.
        own_w = np.where(top2[sel, 0] == e, wsm[sel, 0], wsm[sel, 1])
        sel = sel[np.argsort(-own_w, kind="stable")]
        sels.append(sel)
        counts.append(len(sel))
    # cap needs no partition alignment — tokens are the free dim everywhere.
    # Round to even so the DoubleRow moving pair stays aligned.
    cap = max(NTOK, math.ceil(max(counts) / 2) * 2)

    # ---- Token dispatch (all-to-all equivalent) ----
    in_maps = []
    for e in range(E):
        sel = sels[e]
        cnt = len(sel)
        row = top2[sel]
        tvr = tv[sel]
        own = np.where(row[:, 0] == e, tvr[:, 0], tvr[:, 1])
        other = np.where(row[:, 0] == e, tvr[:, 1], tvr[:, 0])

        # x8: [ci, co, slot(0=lo,1=hi), t]
        xe = np.zeros((P, CO, 2, cap), dtype=E4)
        xt = xf[sel].T.reshape(CO, P, cnt).transpose(1, 0, 2)  # (ci, co, t)
        xhi, xlo = _split8(xt)
        xe[:, :, 0, :cnt] = xlo
        xe[:, :, 1, :cnt] = xhi

        dlg_v = np.full((cap,), -60.0, dtype=np.float32)
        dlg_v[:cnt] = own - other
        dlg_b = np.ascontiguousarray(
            np.broadcast_to(dlg_v[None, :], (P, cap)), dtype=np.float32
        )

        # w1: [ci, co, slot(0=hi,1=lo), f]
        w1t = (W1[e] * S1).reshape(CO, P, F).transpose(1, 0, 2)  # (ci, co, f)
        w1hi, w1lo = _split8(w1t)
        w1e = np.stack([w1hi, w1lo], axis=2)

        # w2: [fi, co, fo, slot(0=hi,1=lo), cc]; w2h: planar hi-only copy.
        w2t = (W2[e] * S2).reshape(FO, P, CO, P).transpose(1, 2, 0, 3)
        w2hi, w2lo = _split8(w2t)                      # (fi, co, fo, cc)
        w2e = np.stack([w2hi, w2lo], axis=3)

        in_maps.append(
            {
                "x8": np.ascontiguousarray(xe),
                "w1": np.ascontiguousarray(w1e),
                "w2": np.ascontiguousarray(w2e),
                "w2h": np.ascontiguousarray(w2hi),
                "dlg": dlg_b,
            }
        )

    # ---- Expert FFN on the 8 NeuronCores ----
    nc = _nc_cache.get(cap)
    if nc is None:
        nc = _nc_cache[cap] = _build_ffn(cap)
    res = run_bass_kernel_spmd(nc, in_maps, core_ids=list(range(E)))

    global last_run_info
    last_run_info = {
        "cap": cap,
        "counts": counts,
        "exec_time_ns": res.exec_time_ns,
        "mean_exec_time_ns": res.mean_exec_time_ns,
        "instructions_and_trace": res.instructions_and_trace,
        "profile_json": res.profile_json,
    }

    # ---- Combine (weighted scatter-add) ----
    out = np.zeros((N, C), dtype=np.float32)
    for e in range(E):
        sel = sels[e]
        out[sel] += res.results[e]["yt"][:, : len(sel)].T
    return out.reshape(B, T, C)


# revision 22
# speedup vs baseline: 1.5101x; 1.0131x over previous
# MoE layer (8 experts, top-2) on 8 TRN2 NeuronCores.
#
# Strategy: expert parallelism (core e owns expert e), per the sharding hint.
#   * Host (control plane): computes gate routing decisions, dispatches
#     ("all-to-all") each token's row to the core(s) owning its top-2 experts,
#     and combines the per-expert partial outputs back into the full output.
#   * Device (data plane): for each core e, computes
#         yT = sigmoid(dlg) * ( W2[e].T @ gelu( W1[e].T @ xT ) )
#     in fp8 (e4m3) DoubleRow perf mode with hi/lo residual compensation.
#
# fp8 DoubleRow: one PE instruction computes lhsT[:,0].T @ rhs[:,0] +
# lhsT[:,1].T @ rhs[:,1] at 0.5 cycles per output row — two fp8 products for
# half the cost of one bf16 k-tile matmul. Every operand O is carried as
# O_hi = fp8(O), O_lo = fp8(O - O_hi), giving ~7 effective mantissa bits.
# Per k-tile pair (A, B) the product (Whi+Wlo)ᵀ(xhi+xlo) is computed as
# three DoubleRow instructions (the lo·lo term is dropped, ~0.07%):
#     main:   [Whi_A, Whi_B] · [xhi_A, xhi_B]
#     crossA: [Whi_A, Wlo_A] · [xlo_A, xhi_A]
#     crossB: [Whi_B, Wlo_B] · [xlo_B, xhi_B]
# = 0.75x the PE cycles of the bf16 kernel with ~0.2% end-to-end error.
#
# Each core's tokens are sorted by their combine weight (descending), and the
# LAST 3 of the 9 token blocks — the ~38% of token-paths with the smallest
# combine weights — drop the x/h (activation) compensation, keeping only the
# weight compensation (main + W_lo cross = 2 products per k-tile, 0.5x bf16).
# Their larger path error (~3.7%) is scaled by small combine weights; the
# measured end-to-end error is ~1.2e-2 against the 2e-2 gate (the numpy
# scheme simulator reproduces the hardware error to 3 decimal places).
#
# Weights are pre-scaled (s1=32, s2=64) so their values sit in e4m3's normal
# range; the scales are undone in the gelu input scale (1/s1) and the
# combine-weight multiply (ce/s2). h = gelu(pre) is split on-chip: the ACT
# engine writes gelu twice (fp8 h_hi and fp32 h), the DVE writes
# h_lo = fp8(h - h_hi).
#
# Dataflow is fully transposed (features on partitions, tokens on the moving
# free dim):
#     phase 1:  hT(F x T)  = W1.T @ xT   (accumulate over C tiles)  -> gelu
#     phase 2:  yT(C x T)  = W2.T @ hT   (accumulate over F tiles)  -> * ce
# W1 (hi+lo fp8, 64 KiB/partition) stays resident in SBUF; W2 streams per
# token block; tokens stream in blocks of 512.

import math

import numpy as np
import ml_dtypes

import concourse.bass as bass
import concourse.mybir as mybir
import concourse.tile as tile
from concourse import bacc
from concourse.bass_utils import run_bass_kernel_spmd

C = 1024          # d_model
F = 4096          # d_ff
E = 8             # experts == cores
P = 128           # SBUF partitions
NTOK = 512        # moving-dim token block (one PSUM bank of fp32)
S1 = 32.0         # W1 pre-scale (W1 ~ N(0, 1/C) -> ~N(0,1))
S2 = 64.0         # W2 pre-scale (W2 ~ N(0, 1/F) -> ~N(0,1))
F8 = mybir.dt.float8e4
F32 = mybir.dt.float32
E4 = ml_dtypes.float8_e4m3
DR = mybir.MatmulPerfMode.DoubleRow

# Filled by kernel() on each call, for the test harness to inspect.
last_run_info: dict = {}

# NEFF-module memo: cap -> compiled Bass module (routing is deterministic in
# the inputs, so repeat calls reuse the same module and its cached NEFF).
_nc_cache: dict = {}


def _block_scheme(b: int, nblk: int) -> str:
    """Compensation scheme per token block (blocks are sorted by combine
    weight, descending): 'F' = full (main + x/W/h crosses), 'W' = weight-only
    (main + W_lo crosses), 'P' = plain (main only)."""
    if b >= nblk - 2:
        return "P"
    if b == nblk - 3:
        return "W"
    return "F"


def _build_ffn(cap: int, ntok: int = NTOK) -> bass.Bass:
    """Per-core expert-FFN kernel (fp8 DoubleRow, hi/lo compensated)."""
    nc = bacc.Bacc()
    CO = C // P   # 8 c-tiles
    FO = F // P   # 32 f-tiles

    # x8: [ci, co, slot, t] with slot 0 = lo, slot 1 = hi (cross pairing
    # needs rhs slots (lo, hi) against lhsT slots (hi, lo)).
    x8 = nc.dram_tensor("x8", [P, CO, 2, cap], F8, kind="ExternalInput")
    # w1: [ci, co, slot, f] with slot 0 = hi, slot 1 = lo.
    w1 = nc.dram_tensor("w1", [P, CO, 2, F], F8, kind="ExternalInput")
    # w2: [fi, co, fo, slot, cc] with slot 0 = hi, slot 1 = lo; each (co)
    # chunk streams as one contiguous 8 KiB-per-partition DMA.
    w2 = nc.dram_tensor("w2", [P, CO, FO, 2, P], F8, kind="ExternalInput")
    # w2h: planar hi-only copy for the plain (main-only) merged pair — a
    # hi-slice of the interleaved layout would stream at 128-byte descriptor
    # granularity (2x DMA latency below 512 B).
    w2h = nc.dram_tensor("w2h", [P, CO, FO, P], F8, kind="ExternalInput")
    dlg = nc.dram_tensor("dlg", [P, cap], F32, kind="ExternalInput")
    yt = nc.dram_tensor("yt", [C, cap], F32, kind="ExternalOutput")

    yt_r = yt.rearrange("(co ci) t -> ci co t", ci=P)

    with tile.TileContext(nc) as tc:
        with (
            tc.tile_pool(name="wts", bufs=1) as wpool,
            tc.tile_pool(name="w2s", bufs=3) as w2pool,
            tc.tile_pool(name="xts", bufs=2) as xpool,
            tc.tile_pool(name="hts", bufs=1) as hpool,
            tc.tile_pool(name="hfs", bufs=4) as hfpool,
            tc.tile_pool(name="ces", bufs=2) as cepool,
            tc.tile_pool(name="yts", bufs=3) as ypool,
            tc.tile_pool(name="ps", bufs=8, space="PSUM") as pspool,
        ):
            # Block 0's token DMAs are issued BEFORE the w1 load: the DMA
            # queue is FIFO and the first matmul needs x8. The hi plane loads
            # first, then the two w1 chunks the first matmul group reads,
            # then x_lo (needed a few matmuls later), then the rest of w1.
            xt0 = xpool.tile([P, CO, 2, ntok], F8, tag="xt")
            t00 = min(ntok, cap)
            nc.sync.dma_start(xt0[:, :, 1, :t00], x8[:, :, 1, :t00])

            # Resident W1 hi+lo (fp8, 64 KiB/partition), loaded in f-major
            # chunks so phase 1's fo-th psum group only waits for the chunk
            # covering it. W2 streams per token block.
            w1_sb = wpool.tile([P, CO, 2, F], F8, tag="w1")
            FCH = 1024
            for co in range(2):
                nc.sync.dma_start(w1_sb[:, co, :, :FCH], w1[:, co, :, :FCH])
            nc.sync.dma_start(xt0[:, :, 0, :t00], x8[:, :, 0, :t00])
            for co in range(2, CO):
                nc.sync.dma_start(w1_sb[:, co, :, :FCH], w1[:, co, :, :FCH])
            for f0 in range(FCH, F, FCH):
                for co in range(CO):
                    nc.sync.dma_start(
                        w1_sb[:, co, :, f0 : f0 + FCH], w1[:, co, :, f0 : f0 + FCH]
                    )

            nblk = (cap + ntok - 1) // ntok
            # The last two blocks merge their phase 2 (one W2 stream for
            # both) when both are plain: the short last block would
            # otherwise consume W2 chunks faster than they stream. Their h_hi
            # planes share one ht tile (slots 0/1), so SBUF is unchanged.
            merge_pair = (
                nblk >= 2
                and _block_scheme(nblk - 2, nblk) == "P"
                and _block_scheme(nblk - 1, nblk) == "P"
            )
            # Phase-1 groups of the next block emitted BEFORE the pending
            # phase 2 (activations deferred after it): the PE chews on them
            # while the previous block's gelu/h_lo tail drains, instead of
            # stalling ~1.2us at every phase1 -> phase2 boundary.
            PK = 3

            def p1_mm(c, fo):
                """Emit phase-1 matmul group fo of block c; return its psum."""
                tn, mode, xt_t = c["tn"], c["mode"], c["xt"]
                fsl = slice(fo * P, (fo + 1) * P)
                ps = pspool.tile([P, ntok], F32, tag="ps")
                for j in range(CO // 2):
                    nc.tensor.matmul(
                        ps[:, :tn],
                        w1_sb[:, 2 * j : 2 * j + 2, 0, fsl],
                        xt_t[:, 2 * j : 2 * j + 2, 1, :tn],
                        start=(j == 0),
                        stop=(mode == "P" and j == CO // 2 - 1),
                        perf_mode=DR,
                    )
                if mode == "F":
                    for co in range(CO):
                        nc.tensor.matmul(
                            ps[:, :tn],
                            w1_sb[:, co, :, fsl],
                            xt_t[:, co, :, :tn],
                            start=False, stop=(co == CO - 1), perf_mode=DR,
                        )
                elif mode == "W":
                    for j in range(CO // 2):
                        nc.tensor.matmul(
                            ps[:, :tn],
                            w1_sb[:, 2 * j : 2 * j + 2, 1, fsl],
                            xt_t[:, 2 * j : 2 * j + 2, 1, :tn],
                            start=False, stop=(j == CO // 2 - 1),
                            perf_mode=DR,
                        )
                return ps

            def p1_act(c, fo, ps):
                """h_hi = fp8(gelu(ps/S1)); for full blocks also the fp32
                gelu (same ACT table, bit-identical inputs) and
                h_lo = fp8(h - h_hi)."""
                tn, ht_t, h_slot = c["tn"], c["ht"], c["slot"]
                nc.scalar.activation(
                    ht_t[:, fo, h_slot, :tn], ps[:, :tn],
                    mybir.ActivationFunctionType.Gelu, scale=1.0 / S1,
                )
                if c["mode"] == "F":
                    hf_t = hfpool.tile([P, ntok], F32, tag="hf")
                    nc.scalar.activation(
                        hf_t[:, :tn], ps[:, :tn],
                        mybir.ActivationFunctionType.Gelu, scale=1.0 / S1,
                    )
                    nc.vector.tensor_tensor(
                        ht_t[:, fo, 0, :tn], hf_t[:, :tn],
                        ht_t[:, fo, 1, :tn], mybir.AluOpType.subtract,
                    )

            def phase2(c):
                """yT = (ce/S2) * (S2 * W2.T @ hT). For the merged (plain)
                pair, each streamed hi-only W2 chunk feeds both blocks'
                psum groups."""
                mode, ht_t = c["mode"], c["ht"]
                groups = c["groups"]
                for co in range(CO):
                    if c["in_pair"]:
                        w2p_t = w2pool.tile([P, FO, P], F8, tag="w2p", bufs=2)
                        nc.sync.dma_start(w2p_t[:], w2h[:, co, :, :])
                        w2_t = None
                    else:
                        w2_t = w2pool.tile([P, FO, 2, P], F8, tag="w2s")
                        nc.sync.dma_start(w2_t[:], w2[:, co, :, :, :])
                    for g_t0, g_tn, g_ce, g_slot in groups:
                        ps2 = pspool.tile([P, ntok], F32, tag="ps")
                        for j in range(FO // 2):
                            lhs_main = (
                                w2p_t[:, 2 * j : 2 * j + 2, :] if c["in_pair"]
                                else w2_t[:, 2 * j : 2 * j + 2, 0, :]
                            )
                            nc.tensor.matmul(
                                ps2[:, :g_tn],
                                lhs_main,
                                ht_t[:, 2 * j : 2 * j + 2, g_slot, :g_tn],
                                start=(j == 0),
                                stop=(mode == "P" and j == FO // 2 - 1),
                                perf_mode=DR,
                            )
                        if mode == "F":
                            for fo in range(FO):
                                nc.tensor.matmul(
                                    ps2[:, :g_tn],
                                    w2_t[:, fo, :, :],
                                    ht_t[:, fo, :, :g_tn],
                                    start=False, stop=(fo == FO - 1),
                                    perf_mode=DR,
                                )
                        elif mode == "W":
                            for j in range(FO // 2):
                                nc.tensor.matmul(
                                    ps2[:, :g_tn],
                                    w2_t[:, 2 * j : 2 * j + 2, 1, :],
                                    ht_t[:, 2 * j : 2 * j + 2, g_slot, :g_tn],
                                    start=False, stop=(j == FO // 2 - 1),
                                    perf_mode=DR,
                                )
                        y_t = ypool.tile([P, ntok], F32, tag="y")
                        nc.vector.tensor_tensor(
                            y_t[:, :g_tn], ps2[:, :g_tn], g_ce[:, :g_tn],
                            mybir.AluOpType.mult,
                        )
                        nc.sync.dma_start(
                            yt_r[:, co, g_t0 : g_t0 + g_tn], y_t[:, :g_tn]
                        )

            ht_pair = None
            pair_state = []
            pend = None
            for b in range(nblk):
                t0 = b * ntok
                tn = min(ntok, cap - t0)
                mode = _block_scheme(b, nblk)
                in_pair = merge_pair and b >= nblk - 2

                if b == 0:
                    xt_t = xt0
                else:
                    xt_t = xpool.tile([P, CO, 2, ntok], F8, tag="xt")
                    if mode == "F":
                        nc.sync.dma_start(
                            xt_t[:, :, :, :tn], x8[:, :, :, t0 : t0 + tn]
                        )
                    else:
                        # W-only/plain blocks never read the x_lo plane.
                        nc.sync.dma_start(
                            xt_t[:, :, 1, :tn], x8[:, :, 1, t0 : t0 + tn]
                        )
                # Combine weight ce = sigmoid(dlg)/S2 = (0.5*tanh(dlg/2)+0.5)/S2
                # (tanh shares an ACT table with gelu; sigmoid does not). The
                # 1/S2 undoes the W2 pre-scale.
                dlg_t = cepool.tile([P, ntok], F32, tag="dlg")
                nc.sync.dma_start(dlg_t[:, :tn], dlg[:, t0 : t0 + tn])
                ce_t = cepool.tile([P, ntok], F32, tag="ce")
                nc.scalar.activation(
                    ce_t[:, :tn], dlg_t[:, :tn],
                    mybir.ActivationFunctionType.Tanh, scale=0.5,
                )
                nc.vector.tensor_scalar(
                    ce_t[:, :tn], ce_t[:, :tn], 0.5 / S2, 0.5 / S2,
                    mybir.AluOpType.mult, mybir.AluOpType.add,
                )

                if in_pair:
                    if ht_pair is None:
                        ht_pair = hpool.tile([P, FO, 2, ntok], F8, tag="ht")
                    ht_t = ht_pair
                    h_slot = b - (nblk - 2)     # 0 for first of pair, 1 for last
                else:
                    ht_t = hpool.tile([P, FO, 2, ntok], F8, tag="ht")
                    h_slot = 1
                c = {
                    "t0": t0, "tn": tn, "mode": mode, "xt": xt_t,
                    "ht": ht_t, "slot": h_slot, "in_pair": in_pair,
                    "groups": None,
                }

                held = []
                if pend is not None:
                    for fo in range(min(PK, FO)):
                        held.append((fo, p1_mm(c, fo)))
                    phase2(pend)
                    pend = None
                for fo, ps in held:
                    p1_act(c, fo, ps)
                for fo in range(len(held), FO):
                    p1_act(c, fo, p1_mm(c, fo))

                if in_pair and b == nblk - 2:
                    # Phase 2 deferred into the merged pass of the last block.
                    pair_state.append((t0, tn, ce_t, h_slot))
                else:
                    c["groups"] = pair_state + [(t0, tn, ce_t, h_slot)]
                    pend = c
            if pend is not None:
                phase2(pend)

    # bacc passes: register allocation, and crucially generate_event_semaphores,
    # which splits multi-wait sync conditions (HW allows 1 wait per instruction).
    nc.compile()

    # Guard: the Tile allocator believes SBUF is 224 KiB/partition (the ISA
    # constant), but exceeding ~192 KiB crashes the TRN2 exec unit. Keep a
    # hard ceiling so overflows fail at build time, not on silicon.
    hw = 0
    for alloc in nc.to_json()["functions"][0]["allocations"]:
        for ml in alloc.get("memorylocations") or []:
            if ml.get("type") == "SB":
                hw = max(hw, ml["addr"] + ml["dims"][1])
    assert hw <= 184 * 1024, f"SBUF high-water {hw / 1024:.1f} KiB exceeds 184 KiB"
    return nc


def _gate_jax_cpu(xf: np.ndarray, Wg: np.ndarray):
    """Reproduce the reference's gate bit-exactly: fp32 matmul + lax.top_k
    on the jax CPU backend (including its tie-breaking). Falls back to a
    numpy gate (correct except possibly on exact fp32 knife-edge ties) if
    jax is unavailable."""
    try:
        import jax

        cpu = jax.devices("cpu")[0]
        with jax.default_device(cpu):
            logits = jax.device_put(xf, cpu) @ jax.device_put(Wg, cpu)
            tv, ti = jax.lax.top_k(logits, 2)
            return np.asarray(ti), np.asarray(tv)
    except Exception:
        logits = xf @ Wg
        part = np.argpartition(-logits, 1, axis=1)[:, :2]
        pv = np.take_along_axis(logits, part, axis=1)
        order = np.argsort(-pv, axis=1, kind="stable")
        ti = np.take_along_axis(part, order, axis=1)
        tv = np.take_along_axis(logits, ti, axis=1)
        return ti, tv


def _split8(a: np.ndarray):
    """hi/lo e4m3 residual split of a float32 array."""
    hi = a.astype(E4)
    lo = (a - hi.astype(np.float32)).astype(E4)
    return hi, lo


def kernel(x, Wg, W1, W2):
    x = np.asarray(x, dtype=np.float32)
    Wg = np.asarray(Wg, dtype=np.float32)
    W1 = np.asarray(W1, dtype=np.float32)
    W2 = np.asarray(W2, dtype=np.float32)

    B, T, _ = x.shape
    N = B * T
    xf = x.reshape(N, C)
    CO, FO = C // P, F // P

    # ---- Gate + routing (control plane) ----
    # Routing decisions are knife-edge sensitive: compute the gate with the
    # same jax-on-CPU ops the reference uses so the top-2 selection matches
    # it bit-for-bit.
    top2, tv = _gate_jax_cpu(xf, Wg)                        # (N, 2) ids / logits

    # Softmax weights for the sort: own = weight of the owning expert.
    wsm = np.exp(tv - tv.max(1, keepdims=True))
    wsm = wsm / wsm.sum(1, keepdims=True)

    sels = []
    counts = []
    for e in range(E):
        sel = np.nonzero((top2 == e).any(axis=1))[0]
        # Sort descending by this expert's combine weight so the trailing
        # (W-only compensated) blocks hold the lowest-stakes token paths.
        own_w = np.where(top2[sel, 0] == e, wsm[sel, 0], wsm[sel, 1])
        sel = sel[np.argsort(-own_w, kind="stable")]
        sels.append(sel)
        counts.append(len(sel))
    # cap needs no partition alignment — tokens are the free dim everywhere.
    # Round to even so the DoubleRow moving pair stays aligned.
    cap = max(NTOK, math.ceil(max(counts) / 2) * 2)

    # ---- Token dispatch (all-to-all equivalent) ----
    in_maps = []
    for e in range(E):
        sel = sels[e]
        cnt = len(sel)
        row = top2[sel]
        tvr = tv[sel]
        own = np.where(row[:, 0] == e, tvr[:, 0], tvr[:, 1])
        other = np.where(row[:, 0] == e, tvr[:, 1], tvr[:, 0])

        # x8: [ci, co, slot(0=lo,1=hi), t]
        xe = np.zeros((P, CO, 2, cap), dtype=E4)
        xt = xf[sel].T.reshape(CO, P, cnt).transpose(1, 0, 2)  # (ci, co, t)
        xhi, xlo = _split8(xt)
        xe[:, :, 0, :cnt] = xlo
        xe[:, :, 1, :cnt] = xhi

        dlg_v = np.full((cap,), -60.0, dtype=np.float32)
        dlg_v[:cnt] = own - other
        dlg_b = np.ascontiguousarray(
            np.broadcast_to(dlg_v[None, :], (P, cap)), dtype=np.float32
        )

        # w1: [ci, co, slot(0=hi,1=lo), f]
        w1t = (W1[e] * S1).reshape(CO, P, F).transpose(1, 0, 2)  # (ci, co, f)
        w1hi, w1lo = _split8(w1t)
        w1e = np.stack([w1hi, w1lo], axis=2)

        # w2: [fi, co, fo, slot(0=hi,1=lo), cc]; w2h: planar hi-only copy.
        w2t = (W2[e] * S2).reshape(FO, P, CO, P).transpose(1, 2, 0, 3)
        w2hi, w2lo = _split8(w2t)                      # (fi, co, fo, cc)
        w2e = np.stack([w2hi, w2lo], axis=3)

        in_maps.append(
            {
                "x8": np.ascontiguousarray(xe),
                "w1": np.ascontiguousarray(w1e),
                "w2": np.ascontiguousarray(w2e),
                "w2h": np.ascontiguousarray(w2hi),
                "dlg": dlg_b,
            }
        )

    # ---- Expert FFN on the 8 NeuronCores ----
    nc = _nc_cache.get(cap)
    if nc is None:
        nc = _nc_cache[cap] = _build_ffn(cap)
    res = run_bass_kernel_spmd(nc, in_maps, core_ids=list(range(E)))

    global last_run_info
    last_run_info = {
        "cap": cap,
        "counts": counts,
        "exec_time_ns": res.exec_time_ns,
        "mean_exec_time_ns": res.mean_exec_time_ns,
        "instructions_and_trace": res.instructions_and_trace,
        "profile_json": res.profile_json,
    }

    # ---- Combine (weighted scatter-add) ----
    out = np.zeros((N, C), dtype=np.float32)
    for e in range(E):
        sel = sels[e]
        out[sel] += res.results[e]["yt"][:, : len(sel)].T
    return out.reshape(B, T, C)


# revision 25
# speedup vs baseline: 1.5172x; 1.0047x over previous
# MoE layer (8 experts, top-2) on 8 TRN2 NeuronCores.
#
# Strategy: expert parallelism (core e owns expert e), per the sharding hint.
#   * Host (control plane): computes gate routing decisions, dispatches
#     ("all-to-all") each token's row to the core(s) owning its top-2 experts,
#     and combines the per-expert partial outputs back into the full output.
#   * Device (data plane): for each core e, computes
#         yT = sigmoid(dlg) * ( W2[e].T @ gelu( W1[e].T @ xT ) )
#     in fp8 (e4m3) DoubleRow perf mode with hi/lo residual compensation.
#
# fp8 DoubleRow: one PE instruction computes lhsT[:,0].T @ rhs[:,0] +
# lhsT[:,1].T @ rhs[:,1] at 0.5 cycles per output row — two fp8 products for
# half the cost of one bf16 k-tile matmul. Every operand O is carried as
# O_hi = fp8(O), O_lo = fp8(O - O_hi), giving ~7 effective mantissa bits.
# Per k-tile pair (A, B) the product (Whi+Wlo)ᵀ(xhi+xlo) is computed as
# three DoubleRow instructions (the lo·lo term is dropped, ~0.07%):
#     main:   [Whi_A, Whi_B] · [xhi_A, xhi_B]
#     crossA: [Whi_A, Wlo_A] · [xlo_A, xhi_A]
#     crossB: [Whi_B, Wlo_B] · [xlo_B, xhi_B]
# = 0.75x the PE cycles of the bf16 kernel with ~0.2% end-to-end error.
#
# Each core's tokens are sorted by their combine weight (descending), and the
# LAST 3 of the 9 token blocks — the ~38% of token-paths with the smallest
# combine weights — drop the x/h (activation) compensation, keeping only the
# weight compensation (main + W_lo cross = 2 products per k-tile, 0.5x bf16).
# Their larger path error (~3.7%) is scaled by small combine weights; the
# measured end-to-end error is ~1.2e-2 against the 2e-2 gate (the numpy
# scheme simulator reproduces the hardware error to 3 decimal places).
#
# Weights are pre-scaled (s1=32, s2=64) so their values sit in e4m3's normal
# range; the scales are undone in the gelu input scale (1/s1) and the
# combine-weight multiply (ce/s2). h = gelu(pre) is split on-chip: the ACT
# engine writes gelu twice (fp8 h_hi and fp32 h), the DVE writes
# h_lo = fp8(h - h_hi).
#
# Dataflow is fully transposed (features on partitions, tokens on the moving
# free dim):
#     phase 1:  hT(F x T)  = W1.T @ xT   (accumulate over C tiles)  -> gelu
#     phase 2:  yT(C x T)  = W2.T @ hT   (accumulate over F tiles)  -> * ce
# W1 (hi+lo fp8, 64 KiB/partition) stays resident in SBUF; W2 streams per
# token block; tokens stream in blocks of 512.

import math

import numpy as np
import ml_dtypes

import concourse.bass as bass
import concourse.mybir as mybir
import concourse.tile as tile
from concourse import bacc
from concourse.bass_utils import run_bass_kernel_spmd

C = 1024          # d_model
F = 4096          # d_ff
E = 8             # experts == cores
P = 128           # SBUF partitions
NTOK = 512        # moving-dim token block (one PSUM bank of fp32)
S1 = 32.0         # W1 pre-scale (W1 ~ N(0, 1/C) -> ~N(0,1))
S2 = 64.0         # W2 pre-scale (W2 ~ N(0, 1/F) -> ~N(0,1))
F8 = mybir.dt.float8e4
BF16 = mybir.dt.bfloat16
F32 = mybir.dt.float32
E4 = ml_dtypes.float8_e4m3
DR = mybir.MatmulPerfMode.DoubleRow

# Filled by kernel() on each call, for the test harness to inspect.
last_run_info: dict = {}

# NEFF-module memo: cap -> compiled Bass module (routing is deterministic in
# the inputs, so repeat calls reuse the same module and its cached NEFF).
_nc_cache: dict = {}


def _block_scheme(b: int, nblk: int) -> str:
    """Compensation scheme per token block (blocks are sorted by combine
    weight, descending): 'F' = full (main + x/W/h crosses), 'W' = weight-only
    (main + W_lo crosses), 'P' = plain (main only)."""
    if b >= nblk - 2:
        return "P"
    if b == nblk - 3:
        return "W"
    return "F"


def _build_ffn(cap: int, ntok: int = NTOK) -> bass.Bass:
    """Per-core expert-FFN kernel (fp8 DoubleRow, hi/lo compensated)."""
    nc = bacc.Bacc()
    CO = C // P   # 8 c-tiles
    FO = F // P   # 32 f-tiles

    # x8: [ci, co, slot, t] with slot 0 = lo, slot 1 = hi (cross pairing
    # needs rhs slots (lo, hi) against lhsT slots (hi, lo)).
    x8 = nc.dram_tensor("x8", [P, CO, 2, cap], F8, kind="ExternalInput")
    # w1: [ci, co, slot, f] with slot 0 = hi, slot 1 = lo.
    w1 = nc.dram_tensor("w1", [P, CO, 2, F], F8, kind="ExternalInput")
    # w2: [fi, co, fo, slot, cc] with slot 0 = hi, slot 1 = lo; each (co)
    # chunk streams as one contiguous 8 KiB-per-partition DMA.
    w2 = nc.dram_tensor("w2", [P, CO, FO, 2, P], F8, kind="ExternalInput")
    # w2h: planar hi-only copy for the plain (main-only) merged pair — a
    # hi-slice of the interleaved layout would stream at 128-byte descriptor
    # granularity (2x DMA latency below 512 B).
    w2h = nc.dram_tensor("w2h", [P, CO, FO, P], F8, kind="ExternalInput")
    dlg = nc.dram_tensor("dlg", [P, cap], F32, kind="ExternalInput")
    yt = nc.dram_tensor("yt", [C, cap], BF16, kind="ExternalOutput")

    yt_r = yt.rearrange("(co ci) t -> ci co t", ci=P)

    with tile.TileContext(nc) as tc:
        with (
            tc.tile_pool(name="wts", bufs=1) as wpool,
            tc.tile_pool(name="w2s", bufs=3) as w2pool,
            tc.tile_pool(name="xts", bufs=2) as xpool,
            tc.tile_pool(name="hts", bufs=1) as hpool,
            tc.tile_pool(name="hfs", bufs=4) as hfpool,
            tc.tile_pool(name="ces", bufs=2) as cepool,
            tc.tile_pool(name="yts", bufs=3) as ypool,
            tc.tile_pool(name="ps", bufs=8, space="PSUM") as pspool,
        ):
            # Block 0's token DMAs are issued BEFORE the w1 load: the DMA
            # queue is FIFO and the first matmul needs x8. The hi plane loads
            # first, then the two w1 chunks the first matmul group reads,
            # then x_lo (needed a few matmuls later), then the rest of w1.
            xt0 = xpool.tile([P, CO, 2, ntok], F8, tag="xt")
            t00 = min(ntok, cap)
            nc.sync.dma_start(xt0[:, :, 1, :t00], x8[:, :, 1, :t00])

            # Resident W1 hi+lo (fp8, 64 KiB/partition), loaded in f-major
            # chunks so phase 1's fo-th psum group only waits for the chunk
            # covering it. W2 streams per token block.
            w1_sb = wpool.tile([P, CO, 2, F], F8, tag="w1")
            FCH = 1024
            for co in range(2):
                nc.sync.dma_start(w1_sb[:, co, :, :FCH], w1[:, co, :, :FCH])
            nc.sync.dma_start(xt0[:, :, 0, :t00], x8[:, :, 0, :t00])
            for co in range(2, CO):
                nc.sync.dma_start(w1_sb[:, co, :, :FCH], w1[:, co, :, :FCH])
            for f0 in range(FCH, F, FCH):
                for co in range(CO):
                    nc.sync.dma_start(
                        w1_sb[:, co, :, f0 : f0 + FCH], w1[:, co, :, f0 : f0 + FCH]
                    )

            nblk = (cap + ntok - 1) // ntok
            # The last two blocks merge their phase 2 (one W2 stream for
            # both) when both are plain: the short last block would
            # otherwise consume W2 chunks faster than they stream. Their h_hi
            # planes share one ht tile (slots 0/1), so SBUF is unchanged.
            merge_pair = (
                nblk >= 2
                and _block_scheme(nblk - 2, nblk) == "P"
                and _block_scheme(nblk - 1, nblk) == "P"
            )
            # Phase-1 groups of the next block emitted BEFORE the pending
            # phase 2 (activations deferred after it): the PE chews on them
            # while the previous block's gelu/h_lo tail drains, instead of
            # stalling ~1.2us at every phase1 -> phase2 boundary.
            PK = 3

            def p1_mm(c, fo):
                """Emit phase-1 matmul group fo of block c; return its psum."""
                tn, mode, xt_t = c["tn"], c["mode"], c["xt"]
                fsl = slice(fo * P, (fo + 1) * P)
                ps = pspool.tile([P, ntok], F32, tag="ps")
                for j in range(CO // 2):
                    nc.tensor.matmul(
                        ps[:, :tn],
                        w1_sb[:, 2 * j : 2 * j + 2, 0, fsl],
                        xt_t[:, 2 * j : 2 * j + 2, 1, :tn],
                        start=(j == 0),
                        stop=(mode == "P" and j == CO // 2 - 1),
                        perf_mode=DR,
                    )
                if mode == "F":
                    for co in range(CO):
                        nc.tensor.matmul(
                            ps[:, :tn],
                            w1_sb[:, co, :, fsl],
                            xt_t[:, co, :, :tn],
                            start=False, stop=(co == CO - 1), perf_mode=DR,
                        )
                elif mode == "W":
                    for j in range(CO // 2):
                        nc.tensor.matmul(
                            ps[:, :tn],
                            w1_sb[:, 2 * j : 2 * j + 2, 1, fsl],
                            xt_t[:, 2 * j : 2 * j + 2, 1, :tn],
                            start=False, stop=(j == CO // 2 - 1),
                            perf_mode=DR,
                        )
                return ps

            def p1_act(c, fo, ps):
                """h_hi = fp8(gelu(ps/S1)); for full blocks also the fp32
                gelu (same ACT table, bit-identical inputs) and
                h_lo = fp8(h - h_hi)."""
                tn, ht_t, h_slot = c["tn"], c["ht"], c["slot"]
                nc.scalar.activation(
                    ht_t[:, fo, h_slot, :tn], ps[:, :tn],
                    mybir.ActivationFunctionType.Gelu, scale=1.0 / S1,
                )
                if c["mode"] == "F":
                    hf_t = hfpool.tile([P, ntok], F32, tag="hf")
                    nc.scalar.activation(
                        hf_t[:, :tn], ps[:, :tn],
                        mybir.ActivationFunctionType.Gelu, scale=1.0 / S1,
                    )
                    nc.vector.tensor_tensor(
                        ht_t[:, fo, 0, :tn], hf_t[:, :tn],
                        ht_t[:, fo, 1, :tn], mybir.AluOpType.subtract,
                    )

            def phase2(c):
                """yT = (ce/S2) * (S2 * W2.T @ hT). For the merged (plain)
                pair, each streamed hi-only W2 chunk feeds both blocks'
                psum groups."""
                mode, ht_t = c["mode"], c["ht"]
                groups = c["groups"]
                for co in range(CO):
                    if c["in_pair"]:
                        w2p_t = w2pool.tile([P, FO, P], F8, tag="w2p", bufs=2)
                        nc.sync.dma_start(w2p_t[:], w2h[:, co, :, :])
                        w2_t = None
                    else:
                        w2_t = w2pool.tile([P, FO, 2, P], F8, tag="w2s")
                        nc.sync.dma_start(w2_t[:], w2[:, co, :, :, :])
                    for g_t0, g_tn, g_ce, g_slot in groups:
                        ps2 = pspool.tile([P, ntok], F32, tag="ps")
                        for j in range(FO // 2):
                            lhs_main = (
                                w2p_t[:, 2 * j : 2 * j + 2, :] if c["in_pair"]
                                else w2_t[:, 2 * j : 2 * j + 2, 0, :]
                            )
                            nc.tensor.matmul(
                                ps2[:, :g_tn],
                                lhs_main,
                                ht_t[:, 2 * j : 2 * j + 2, g_slot, :g_tn],
                                start=(j == 0),
                                stop=(mode == "P" and j == FO // 2 - 1),
                                perf_mode=DR,
                            )
                        if mode == "F":
                            for fo in range(FO):
                                nc.tensor.matmul(
                                    ps2[:, :g_tn],
                                    w2_t[:, fo, :, :],
                                    ht_t[:, fo, :, :g_tn],
                                    start=False, stop=(fo == FO - 1),
                                    perf_mode=DR,
                                )
                        elif mode == "W":
                            for j in range(FO // 2):
                                nc.tensor.matmul(
                                    ps2[:, :g_tn],
                                    w2_t[:, 2 * j : 2 * j + 2, 1, :],
                                    ht_t[:, 2 * j : 2 * j + 2, g_slot, :g_tn],
                                    start=False, stop=(j == FO // 2 - 1),
                                    perf_mode=DR,
                                )
                        y_t = ypool.tile([P, ntok], BF16, tag="y")
                        nc.vector.tensor_tensor(
                            y_t[:, :g_tn], ps2[:, :g_tn], g_ce[:, :g_tn],
                            mybir.AluOpType.mult,
                        )
                        nc.sync.dma_start(
                            yt_r[:, co, g_t0 : g_t0 + g_tn], y_t[:, :g_tn]
                        )

            ht_pair = None
            pair_state = []
            pend = None
            for b in range(nblk):
                t0 = b * ntok
                tn = min(ntok, cap - t0)
                mode = _block_scheme(b, nblk)
                in_pair = merge_pair and b >= nblk - 2

                if b == 0:
                    xt_t = xt0
                else:
                    xt_t = xpool.tile([P, CO, 2, ntok], F8, tag="xt")
                    if mode == "F":
                        nc.sync.dma_start(
                            xt_t[:, :, :, :tn], x8[:, :, :, t0 : t0 + tn]
                        )
                    else:
                        # W-only/plain blocks never read the x_lo plane.
                        nc.sync.dma_start(
                            xt_t[:, :, 1, :tn], x8[:, :, 1, t0 : t0 + tn]
                        )
                # Combine weight ce = sigmoid(dlg)/S2 = (0.5*tanh(dlg/2)+0.5)/S2
                # (tanh shares an ACT table with gelu; sigmoid does not). The
                # 1/S2 undoes the W2 pre-scale.
                dlg_t = cepool.tile([P, ntok], F32, tag="dlg")
                nc.sync.dma_start(dlg_t[:, :tn], dlg[:, t0 : t0 + tn])
                ce_t = cepool.tile([P, ntok], F32, tag="ce")
                nc.scalar.activation(
                    ce_t[:, :tn], dlg_t[:, :tn],
                    mybir.ActivationFunctionType.Tanh, scale=0.5,
                )
                nc.vector.tensor_scalar(
                    ce_t[:, :tn], ce_t[:, :tn], 0.5 / S2, 0.5 / S2,
                    mybir.AluOpType.mult, mybir.AluOpType.add,
                )

                if in_pair:
                    if ht_pair is None:
                        ht_pair = hpool.tile([P, FO, 2, ntok], F8, tag="ht")
                    ht_t = ht_pair
                    h_slot = b - (nblk - 2)     # 0 for first of pair, 1 for last
                else:
                    ht_t = hpool.tile([P, FO, 2, ntok], F8, tag="ht")
                    h_slot = 1
                c = {
                    "t0": t0, "tn": tn, "mode": mode, "xt": xt_t,
                    "ht": ht_t, "slot": h_slot, "in_pair": in_pair,
                    "groups": None,
                }

                held = []
                if pend is not None:
                    # Prefix-pipeline only across Full-block boundaries: their
                    # gelu/h_lo tail is the long one, and skipping the prefix
                    # for W/P predecessors keeps this block's x8/dlg loads out
                    # of the W-block's DMA-saturated phase-2 window.
                    if pend["mode"] == "F":
                        for fo in range(min(PK, FO)):
                            held.append((fo, p1_mm(c, fo)))
                    phase2(pend)
                    pend = None
                for fo, ps in held:
                    p1_act(c, fo, ps)
                for fo in range(len(held), FO):
                    p1_act(c, fo, p1_mm(c, fo))

                if in_pair and b == nblk - 2:
                    # Phase 2 deferred into the merged pass of the last block.
                    pair_state.append((t0, tn, ce_t, h_slot))
                else:
                    c["groups"] = pair_state + [(t0, tn, ce_t, h_slot)]
                    pend = c
            if pend is not None:
                phase2(pend)

    # bacc passes: register allocation, and crucially generate_event_semaphores,
    # which splits multi-wait sync conditions (HW allows 1 wait per instruction).
    nc.compile()

    # Guard: the Tile allocator believes SBUF is 224 KiB/partition (the ISA
    # constant), but exceeding ~192 KiB crashes the TRN2 exec unit. Keep a
    # hard ceiling so overflows fail at build time, not on silicon.
    hw = 0
    for alloc in nc.to_json()["functions"][0]["allocations"]:
        for ml in alloc.get("memorylocations") or []:
            if ml.get("type") == "SB":
                hw = max(hw, ml["addr"] + ml["dims"][1])
    assert hw <= 184 * 1024, f"SBUF high-water {hw / 1024:.1f} KiB exceeds 184 KiB"
    return nc


def _gate_jax_cpu(xf: np.ndarray, Wg: np.ndarray):
    """Reproduce the reference's gate bit-exactly: fp32 matmul + lax.top_k
    on the jax CPU backend (including its tie-breaking). Falls back to a
    numpy gate (correct except possibly on exact fp32 knife-edge ties) if
    jax is unavailable."""
    try:
        import jax

        cpu = jax.devices("cpu")[0]
        with jax.default_device(cpu):
            logits = jax.device_put(xf, cpu) @ jax.device_put(Wg, cpu)
            tv, ti = jax.lax.top_k(logits, 2)
            return np.asarray(ti), np.asarray(tv)
    except Exception:
        logits = xf @ Wg
        part = np.argpartition(-logits, 1, axis=1)[:, :2]
        pv = np.take_along_axis(logits, part, axis=1)
        order = np.argsort(-pv, axis=1, kind="stable")
        ti = np.take_along_axis(part, order, axis=1)
        tv = np.take_along_axis(logits, ti, axis=1)
        return ti, tv


def _split8(a: np.ndarray):
    """hi/lo e4m3 residual split of a float32 array."""
    hi = a.astype(E4)
    lo = (a - hi.astype(np.float32)).astype(E4)
    return hi, lo


def kernel(x, Wg, W1, W2):
    x = np.asarray(x, dtype=np.float32)
    Wg = np.asarray(Wg, dtype=np.float32)
    W1 = np.asarray(W1, dtype=np.float32)
    W2 = np.asarray(W2, dtype=np.float32)

    B, T, _ = x.shape
    N = B * T
    xf = x.reshape(N, C)
    CO, FO = C // P, F // P

    # ---- Gate + routing (control plane) ----
    # Routing decisions are knife-edge sensitive: compute the gate with the
    # same jax-on-CPU ops the reference uses so the top-2 selection matches
    # it bit-for-bit.
    top2, tv = _gate_jax_cpu(xf, Wg)                        # (N, 2) ids / logits

    # Softmax weights for the sort: own = weight of the owning expert.
    wsm = np.exp(tv - tv.max(1, keepdims=True))
    wsm = wsm / wsm.sum(1, keepdims=True)

    sels = []
    counts = []
    for e in range(E):
        sel = np.nonzero((top2 == e).any(axis=1))[0]
        # Sort descending by this expert's combine weight so the trailing
        # (W-only compensated) blocks hold the lowest-stakes token paths.
        own_w = np.where(top2[sel, 0] == e, wsm[sel, 0], wsm[sel, 1])
        sel = sel[np.argsort(-own_w, kind="stable")]
        sels.append(sel)
        counts.append(len(sel))
    # cap needs no partition alignment — tokens are the free dim everywhere.
    # Round to even so the DoubleRow moving pair stays aligned.
    cap = max(NTOK, math.ceil(max(counts) / 2) * 2)

    # ---- Token dispatch (all-to-all equivalent) ----
    in_maps = []
    for e in range(E):
        sel = sels[e]
        cnt = len(sel)
        row = top2[sel]
        tvr = tv[sel]
        own = np.where(row[:, 0] == e, tvr[:, 0], tvr[:, 1])
        other = np.where(row[:, 0] == e, tvr[:, 1], tvr[:, 0])

        # x8: [ci, co, slot(0=lo,1=hi), t]
        xe = np.zeros((P, CO, 2, cap), dtype=E4)
        xt = xf[sel].T.reshape(CO, P, cnt).transpose(1, 0, 2)  # (ci, co, t)
        xhi, xlo = _split8(xt)
        xe[:, :, 0, :cnt] = xlo
        xe[:, :, 1, :cnt] = xhi

        dlg_v = np.full((cap,), -60.0, dtype=np.float32)
        dlg_v[:cnt] = own - other
        dlg_b = np.ascontiguousarray(
            np.broadcast_to(dlg_v[None, :], (P, cap)), dtype=np.float32
        )

        # w1: [ci, co, slot(0=hi,1=lo), f]
        w1t = (W1[e] * S1).reshape(CO, P, F).transpose(1, 0, 2)  # (ci, co, f)
        w1hi, w1lo = _split8(w1t)
        w1e = np.stack([w1hi, w1lo], axis=2)

        # w2: [fi, co, fo, slot(0=hi,1=lo), cc]; w2h: planar hi-only copy.
        w2t = (W2[e] * S2).reshape(FO, P, CO, P).transpose(1, 2, 0, 3)
        w2hi, w2lo = _split8(w2t)                      # (fi, co, fo, cc)
        w2e = np.stack([w2hi, w2lo], axis=3)

        in_maps.append(
            {
                "x8": np.ascontiguousarray(xe),
                "w1": np.ascontiguousarray(w1e),
                "w2": np.ascontiguousarray(w2e),
                "w2h": np.ascontiguousarray(w2hi),
                "dlg": dlg_b,
            }
        )

    # ---- Expert FFN on the 8 NeuronCores ----
    nc = _nc_cache.get(cap)
    if nc is None:
        nc = _nc_cache[cap] = _build_ffn(cap)
    res = run_bass_kernel_spmd(nc, in_maps, core_ids=list(range(E)))

    global last_run_info
    last_run_info = {
        "cap": cap,
        "counts": counts,
        "exec_time_ns": res.exec_time_ns,
        "mean_exec_time_ns": res.mean_exec_time_ns,
        "instructions_and_trace": res.instructions_and_trace,
        "profile_json": res.profile_json,
    }

    # ---- Combine (weighted scatter-add) ----
    out = np.zeros((N, C), dtype=np.float32)
    for e in range(E):
        sel = sels[e]
        out[sel] += res.results[e]["yt"][:, : len(sel)].T.astype(np.float32)
    return out.reshape(B, T, C)


# revision 29
# speedup vs baseline: 1.5342x; 1.0112x over previous
# MoE layer (8 experts, top-2) on 8 TRN2 NeuronCores.
#
# Strategy: expert parallelism (core e owns expert e), per the sharding hint.
#   * Host (control plane): computes gate routing decisions, dispatches
#     ("all-to-all") each token's row to the core(s) owning its top-2 experts,
#     and combines the per-expert partial outputs back into the full output.
#   * Device (data plane): for each core e, computes
#         yT = sigmoid(dlg) * ( W2[e].T @ gelu( W1[e].T @ xT ) )
#     in fp8 (e4m3) DoubleRow perf mode with hi/lo residual compensation.
#
# fp8 DoubleRow: one PE instruction computes lhsT[:,0].T @ rhs[:,0] +
# lhsT[:,1].T @ rhs[:,1] at 0.5 cycles per output row — two fp8 products for
# half the cost of one bf16 k-tile matmul. Every operand O is carried as
# O_hi = fp8(O), O_lo = fp8(O - O_hi), giving ~7 effective mantissa bits.
# Per k-tile pair (A, B) the product (Whi+Wlo)ᵀ(xhi+xlo) is computed as
# three DoubleRow instructions (the lo·lo term is dropped, ~0.07%):
#     main:   [Whi_A, Whi_B] · [xhi_A, xhi_B]
#     crossA: [Whi_A, Wlo_A] · [xlo_A, xhi_A]
#     crossB: [Whi_B, Wlo_B] · [xlo_B, xhi_B]
# = 0.75x the PE cycles of the bf16 kernel with ~0.2% end-to-end error.
#
# Each core's tokens are sorted by their combine weight (descending), and the
# LAST 3 of the 9 token blocks — the ~38% of token-paths with the smallest
# combine weights — drop the x/h (activation) compensation, keeping only the
# weight compensation (main + W_lo cross = 2 products per k-tile, 0.5x bf16).
# Their larger path error (~3.7%) is scaled by small combine weights; the
# measured end-to-end error is ~1.2e-2 against the 2e-2 gate (the numpy
# scheme simulator reproduces the hardware error to 3 decimal places).
#
# Weights are pre-scaled (s1=32, s2=64) so their values sit in e4m3's normal
# range; the scales are undone in the gelu input scale (1/s1) and the
# combine-weight multiply (ce/s2). h = gelu(pre) is split on-chip: the ACT
# engine writes gelu twice (fp8 h_hi and fp32 h), the DVE writes
# h_lo = fp8(h - h_hi).
#
# Dataflow is fully transposed (features on partitions, tokens on the moving
# free dim):
#     phase 1:  hT(F x T)  = W1.T @ xT   (accumulate over C tiles)  -> gelu
#     phase 2:  yT(C x T)  = W2.T @ hT   (accumulate over F tiles)  -> * ce
# W1 (hi+lo fp8, 64 KiB/partition) stays resident in SBUF; W2 streams per
# token block; tokens stream in blocks of 512.

import math

import numpy as np
import ml_dtypes

import concourse.bass as bass
import concourse.mybir as mybir
import concourse.tile as tile
from concourse import bacc
from concourse.bass_utils import run_bass_kernel_spmd

C = 1024          # d_model
F = 4096          # d_ff
E = 8             # experts == cores
P = 128           # SBUF partitions
NTOK = 512        # moving-dim token block (one PSUM bank of fp32)
S1 = 32.0         # W1 pre-scale (W1 ~ N(0, 1/C) -> ~N(0,1))
S2 = 64.0         # W2 pre-scale (W2 ~ N(0, 1/F) -> ~N(0,1))
F8 = mybir.dt.float8e4
BF16 = mybir.dt.bfloat16
F32 = mybir.dt.float32
E4 = ml_dtypes.float8_e4m3
DR = mybir.MatmulPerfMode.DoubleRow

# Filled by kernel() on each call, for the test harness to inspect.
last_run_info: dict = {}

# NEFF-module memo: cap -> compiled Bass module (routing is deterministic in
# the inputs, so repeat calls reuse the same module and its cached NEFF).
_nc_cache: dict = {}


def _block_scheme(b: int, nblk: int) -> str:
    """Compensation scheme per token block (blocks are sorted by combine
    weight, descending): 'F' = full (main + x/W/h crosses), 'W' = weight-only
    (main + W_lo crosses), 'P' = plain (main only)."""
    if b >= nblk - 2:
        return "P"
    if b == nblk - 3:
        return "W"
    return "F"


def _build_ffn(cap: int, ntok: int = NTOK) -> bass.Bass:
    """Per-core expert-FFN kernel (fp8 DoubleRow, hi/lo compensated)."""
    nc = bacc.Bacc()
    CO = C // P   # 8 c-tiles
    FO = F // P   # 32 f-tiles

    # x8: [ci, co, slot, t] with slot 0 = lo, slot 1 = hi (cross pairing
    # needs rhs slots (lo, hi) against lhsT slots (hi, lo)).
    x8 = nc.dram_tensor("x8", [P, CO, 2, cap], F8, kind="ExternalInput")
    # w1: [ci, co, slot, f] with slot 0 = hi, slot 1 = lo.
    w1 = nc.dram_tensor("w1", [P, CO, 2, F], F8, kind="ExternalInput")
    # w2: [fi, co, fo, slot, cc] with slot 0 = hi, slot 1 = lo; each (co)
    # chunk streams as one contiguous 8 KiB-per-partition DMA.
    w2 = nc.dram_tensor("w2", [P, CO, FO, 2, P], F8, kind="ExternalInput")
    dlg = nc.dram_tensor("dlg", [P, cap], F32, kind="ExternalInput")
    yt = nc.dram_tensor("yt", [C, cap], BF16, kind="ExternalOutput")

    yt_r = yt.rearrange("(co ci) t -> ci co t", ci=P)

    with tile.TileContext(nc) as tc:
        with (
            tc.tile_pool(name="wts", bufs=1) as wpool,
            tc.tile_pool(name="w2s", bufs=3) as w2pool,
            tc.tile_pool(name="xts", bufs=2) as xpool,
            tc.tile_pool(name="hts", bufs=1) as hpool,
            tc.tile_pool(name="hfs", bufs=2) as hfpool,
            tc.tile_pool(name="ces", bufs=2) as cepool,
            tc.tile_pool(name="yts", bufs=3) as ypool,
            tc.tile_pool(name="ps", bufs=8, space="PSUM") as pspool,
        ):
            # Block 0's token DMAs are issued BEFORE the w1 load: the DMA
            # queue is FIFO and the first matmul needs x8. The hi plane loads
            # first, then the two w1 chunks the first matmul group reads,
            # then x_lo (needed a few matmuls later), then the rest of w1.
            xt0 = xpool.tile([P, CO, 2, ntok], F8, tag="xt")
            t00 = min(ntok, cap)
            nc.sync.dma_start(xt0[:, :, 1, :t00], x8[:, :, 1, :t00])

            # Resident W1 hi+lo (fp8, 64 KiB/partition), loaded in f-major
            # chunks so phase 1's fo-th psum group only waits for the chunk
            # covering it. W2 streams per token block.
            w1_sb = wpool.tile([P, CO, 2, F], F8, tag="w1")
            FCH = 1024
            for co in range(2):
                nc.sync.dma_start(w1_sb[:, co, :, :FCH], w1[:, co, :, :FCH])
            nc.sync.dma_start(xt0[:, :, 0, :t00], x8[:, :, 0, :t00])
            for co in range(2, CO):
                nc.sync.dma_start(w1_sb[:, co, :, :FCH], w1[:, co, :, :FCH])
            for f0 in range(FCH, F, FCH):
                for co in range(CO):
                    nc.sync.dma_start(
                        w1_sb[:, co, :, f0 : f0 + FCH], w1[:, co, :, f0 : f0 + FCH]
                    )

            nblk = (cap + ntok - 1) // ntok
            # All trailing non-Full blocks (W + plain) merge their phase 2
            # into ONE pass: each streamed interleaved W2 chunk feeds every
            # trailing block's psum group, so the chunk cadence (~6us) stays
            # far above its ~2.8us transfer time — the separate W-block and
            # short-last-block phase-2 windows would each starve the DMA.
            # The trailing blocks only write h_hi planes: two share the
            # 2-slot ht tile, the third uses a 16 KiB hi-only tile.
            trail = [b for b in range(nblk) if _block_scheme(b, nblk) != "F"]
            # Phase-1 groups of the next block emitted BEFORE the pending
            # phase 2 (activations deferred after it): the PE chews on them
            # while the previous block's gelu/h_lo tail drains, instead of
            # stalling ~1.2us at every phase1 -> phase2 boundary.
            PK = 3

            def p1_mm(c, fo):
                """Emit phase-1 matmul group fo of block c; return its psum."""
                tn, mode, xt_t = c["tn"], c["mode"], c["xt"]
                fsl = slice(fo * P, (fo + 1) * P)
                ps = pspool.tile([P, ntok], F32, tag="ps")
                for j in range(CO // 2):
                    nc.tensor.matmul(
                        ps[:, :tn],
                        w1_sb[:, 2 * j : 2 * j + 2, 0, fsl],
                        xt_t[:, 2 * j : 2 * j + 2, 1, :tn],
                        start=(j == 0),
                        stop=(mode == "P" and j == CO // 2 - 1),
                        perf_mode=DR,
                    )
                if mode == "F":
                    for co in range(CO):
                        nc.tensor.matmul(
                            ps[:, :tn],
                            w1_sb[:, co, :, fsl],
                            xt_t[:, co, :, :tn],
                            start=False, stop=(co == CO - 1), perf_mode=DR,
                        )
                elif mode == "W":
                    for j in range(CO // 2):
                        nc.tensor.matmul(
                            ps[:, :tn],
                            w1_sb[:, 2 * j : 2 * j + 2, 1, fsl],
                            xt_t[:, 2 * j : 2 * j + 2, 1, :tn],
                            start=False, stop=(j == CO // 2 - 1),
                            perf_mode=DR,
                        )
                return ps

            def p1_act(c, fo, ps):
                """h_hi = fp8(gelu(ps/S1)); for full blocks also the fp32
                gelu (same ACT table, bit-identical inputs) and
                h_lo = fp8(h - h_hi)."""
                tn, ht_t, h_slot = c["tn"], c["ht"], c["slot"]
                nc.scalar.activation(
                    ht_t[:, fo, h_slot, :tn], ps[:, :tn],
                    mybir.ActivationFunctionType.Gelu, scale=1.0 / S1,
                )
                if c["mode"] == "F":
                    hf_t = hfpool.tile([P, ntok], F32, tag="hf")
                    nc.scalar.activation(
                        hf_t[:, :tn], ps[:, :tn],
                        mybir.ActivationFunctionType.Gelu, scale=1.0 / S1,
                    )
                    nc.vector.tensor_tensor(
                        ht_t[:, fo, 0, :tn], hf_t[:, :tn],
                        ht_t[:, fo, 1, :tn], mybir.AluOpType.subtract,
                    )

            def phase2(groups):
                """yT = (ce/S2) * (S2 * W2.T @ hT). Each streamed interleaved
                W2 chunk feeds every group's psum accumulation."""
                for co in range(CO):
                    w2_t = w2pool.tile([P, FO, 2, P], F8, tag="w2s")
                    nc.sync.dma_start(w2_t[:], w2[:, co, :, :, :])
                    for g in groups:
                        g_tn, g_t0, g_ce = g["tn"], g["t0"], g["ce"]
                        ht_t, g_slot, mode = g["ht"], g["slot"], g["mode"]
                        ps2 = pspool.tile([P, ntok], F32, tag="ps")
                        for j in range(FO // 2):
                            nc.tensor.matmul(
                                ps2[:, :g_tn],
                                w2_t[:, 2 * j : 2 * j + 2, 0, :],
                                ht_t[:, 2 * j : 2 * j + 2, g_slot, :g_tn],
                                start=(j == 0),
                                stop=(mode == "P" and j == FO // 2 - 1),
                                perf_mode=DR,
                            )
                        if mode == "F":
                            for fo in range(FO):
                                nc.tensor.matmul(
                                    ps2[:, :g_tn],
                                    w2_t[:, fo, :, :],
                                    ht_t[:, fo, :, :g_tn],
                                    start=False, stop=(fo == FO - 1),
                                    perf_mode=DR,
                                )
                        elif mode == "W":
                            for j in range(FO // 2):
                                nc.tensor.matmul(
                                    ps2[:, :g_tn],
                                    w2_t[:, 2 * j : 2 * j + 2, 1, :],
                                    ht_t[:, 2 * j : 2 * j + 2, g_slot, :g_tn],
                                    start=False, stop=(j == FO // 2 - 1),
                                    perf_mode=DR,
                                )
                        y_t = ypool.tile([P, ntok], BF16, tag="y")
                        nc.vector.tensor_tensor(
                            y_t[:, :g_tn], ps2[:, :g_tn], g_ce[:, :g_tn],
                            mybir.AluOpType.mult,
                        )
                        nc.sync.dma_start(
                            yt_r[:, co, g_t0 : g_t0 + g_tn], y_t[:, :g_tn]
                        )

            ht_trail = None          # 2-slot ht shared by the first two
            trail_state = []         # trailing blocks awaiting merged phase 2
            pend = None
            for b in range(nblk):
                t0 = b * ntok
                tn = min(ntok, cap - t0)
                mode = _block_scheme(b, nblk)
                in_trail = b in trail

                if b == 0:
                    xt_t = xt0
                else:
                    xt_t = xpool.tile([P, CO, 2, ntok], F8, tag="xt")
                    if mode == "F":
                        nc.sync.dma_start(
                            xt_t[:, :, :, :tn], x8[:, :, :, t0 : t0 + tn]
                        )
                    else:
                        # W-only/plain blocks never read the x_lo plane.
                        nc.sync.dma_start(
                            xt_t[:, :, 1, :tn], x8[:, :, 1, t0 : t0 + tn]
                        )
                # Combine weight ce = sigmoid(dlg)/S2 = (0.5*tanh(dlg/2)+0.5)/S2
                # (tanh shares an ACT table with gelu; sigmoid does not). The
                # 1/S2 undoes the W2 pre-scale.
                dlg_t = cepool.tile([P, ntok], F32, tag="dlg", bufs=1)
                nc.sync.dma_start(dlg_t[:, :tn], dlg[:, t0 : t0 + tn])
                ce_t = cepool.tile([P, ntok], F32, tag="ce", bufs=3)
                nc.scalar.activation(
                    ce_t[:, :tn], dlg_t[:, :tn],
                    mybir.ActivationFunctionType.Tanh, scale=0.5,
                )
                nc.vector.tensor_scalar(
                    ce_t[:, :tn], ce_t[:, :tn], 0.5 / S2, 0.5 / S2,
                    mybir.AluOpType.mult, mybir.AluOpType.add,
                )

                if in_trail:
                    ti = trail.index(b)
                    if ti < 2:
                        if ht_trail is None:
                            ht_trail = hpool.tile([P, FO, 2, ntok], F8, tag="ht")
                        ht_t, h_slot = ht_trail, ti
                    else:
                        ht_t = hpool.tile([P, FO, 1, ntok], F8, tag="htp")
                        h_slot = 0
                else:
                    ht_t = hpool.tile([P, FO, 2, ntok], F8, tag="ht")
                    h_slot = 1
                c = {
                    "t0": t0, "tn": tn, "mode": mode, "xt": xt_t,
                    "ht": ht_t, "slot": h_slot, "ce": ce_t,
                }

                held = []
                if pend is not None:
                    # Prefix-pipeline across Full-block boundaries: their
                    # gelu/h_lo tail is the long one.
                    if pend[0]["mode"] == "F":
                        for fo in range(min(PK, FO)):
                            held.append((fo, p1_mm(c, fo)))
                    phase2(pend)
                    pend = None
                for fo, ps in held:
                    p1_act(c, fo, ps)
                for fo in range(len(held), FO):
                    p1_act(c, fo, p1_mm(c, fo))

                if in_trail:
                    # Phase 2 merged across all trailing blocks at the end.
                    trail_state.append(c)
                else:
                    pend = [c]
            if pend is not None:
                phase2(pend)
            if trail_state:
                phase2(trail_state)

    # bacc passes: register allocation, and crucially generate_event_semaphores,
    # which splits multi-wait sync conditions (HW allows 1 wait per instruction).
    nc.compile()

    # Guard: the Tile allocator believes SBUF is 224 KiB/partition (the ISA
    # constant), but exceeding ~192 KiB crashes the TRN2 exec unit. Keep a
    # hard ceiling so overflows fail at build time, not on silicon.
    hw = 0
    for alloc in nc.to_json()["functions"][0]["allocations"]:
        for ml in alloc.get("memorylocations") or []:
            if ml.get("type") == "SB":
                hw = max(hw, ml["addr"] + ml["dims"][1])
    assert hw <= 184 * 1024, f"SBUF high-water {hw / 1024:.1f} KiB exceeds 184 KiB"
    return nc


def _gate_jax_cpu(xf: np.ndarray, Wg: np.ndarray):
    """Reproduce the reference's gate bit-exactly: fp32 matmul + lax.top_k
    on the jax CPU backend (including its tie-breaking). Falls back to a
    numpy gate (correct except possibly on exact fp32 knife-edge ties) if
    jax is unavailable."""
    try:
        import jax

        cpu = jax.devices("cpu")[0]
        with jax.default_device(cpu):
            logits = jax.device_put(xf, cpu) @ jax.device_put(Wg, cpu)
            tv, ti = jax.lax.top_k(logits, 2)
            return np.asarray(ti), np.asarray(tv)
    except Exception:
        logits = xf @ Wg
        part = np.argpartition(-logits, 1, axis=1)[:, :2]
        pv = np.take_along_axis(logits, part, axis=1)
        order = np.argsort(-pv, axis=1, kind="stable")
        ti = np.take_along_axis(part, order, axis=1)
        tv = np.take_along_axis(logits, ti, axis=1)
        return ti, tv


def _split8(a: np.ndarray):
    """hi/lo e4m3 residual split of a float32 array."""
    hi = a.astype(E4)
    lo = (a - hi.astype(np.float32)).astype(E4)
    return hi, lo


def kernel(x, Wg, W1, W2):
    x = np.asarray(x, dtype=np.float32)
    Wg = np.asarray(Wg, dtype=np.float32)
    W1 = np.asarray(W1, dtype=np.float32)
    W2 = np.asarray(W2, dtype=np.float32)

    B, T, _ = x.shape
    N = B * T
    xf = x.reshape(N, C)
    CO, FO = C // P, F // P

    # ---- Gate + routing (control plane) ----
    # Routing decisions are knife-edge sensitive: compute the gate with the
    # same jax-on-CPU ops the reference uses so the top-2 selection matches
    # it bit-for-bit.
    top2, tv = _gate_jax_cpu(xf, Wg)                        # (N, 2) ids / logits

    # Softmax weights for the sort: own = weight of the owning expert.
    wsm = np.exp(tv - tv.max(1, keepdims=True))
    wsm = wsm / wsm.sum(1, keepdims=True)

    sels = []
    counts = []
    for e in range(E):
        sel = np.nonzero((top2 == e).any(axis=1))[0]
        # Sort descending by this expert's combine weight so the trailing
        # (W-only compensated) blocks hold the lowest-stakes token paths.
        own_w = np.where(top2[sel, 0] == e, wsm[sel, 0], wsm[sel, 1])
        sel = sel[np.argsort(-own_w, kind="stable")]
        sels.append(sel)
        counts.append(len(sel))
    # cap needs no partition alignment — tokens are the free dim everywhere.
    # Round to even so the DoubleRow moving pair stays aligned.
    cap = max(NTOK, math.ceil(max(counts) / 2) * 2)

    # ---- Token dispatch (all-to-all equivalent) ----
    in_maps = []
    for e in range(E):
        sel = sels[e]
        cnt = len(sel)
        row = top2[sel]
        tvr = tv[sel]
        own = np.where(row[:, 0] == e, tvr[:, 0], tvr[:, 1])
        other = np.where(row[:, 0] == e, tvr[:, 1], tvr[:, 0])

        # x8: [ci, co, slot(0=lo,1=hi), t]
        xe = np.zeros((P, CO, 2, cap), dtype=E4)
        xt = xf[sel].T.reshape(CO, P, cnt).transpose(1, 0, 2)  # (ci, co, t)
        xhi, xlo = _split8(xt)
        xe[:, :, 0, :cnt] = xlo
        xe[:, :, 1, :cnt] = xhi

        dlg_v = np.full((cap,), -60.0, dtype=np.float32)
        dlg_v[:cnt] = own - other
        dlg_b = np.ascontiguousarray(
            np.broadcast_to(dlg_v[None, :], (P, cap)), dtype=np.float32
        )

        # w1: [ci, co, slot(0=hi,1=lo), f]
        w1t = (W1[e] * S1).reshape(CO, P, F).transpose(1, 0, 2)  # (ci, co, f)
        w1hi, w1lo = _split8(w1t)
        w1e = np.stack([w1hi, w1lo], axis=2)

        # w2: [fi, co, fo, slot(0=hi,1=lo), cc]
        w2t = (W2[e] * S2).reshape(FO, P, CO, P).transpose(1, 2, 0, 3)
        w2hi, w2lo = _split8(w2t)                      # (fi, co, fo, cc)
        w2e = np.stack([w2hi, w2lo], axis=3)

        in_maps.append(
            {
                "x8": np.ascontiguousarray(xe),
                "w1": np.ascontiguousarray(w1e),
                "w2": np.ascontiguousarray(w2e),
                "dlg": dlg_b,
            }
        )

    # ---- Expert FFN on the 8 NeuronCores ----
    nc = _nc_cache.get(cap)
    if nc is None:
        nc = _nc_cache[cap] = _build_ffn(cap)
    res = run_bass_kernel_spmd(nc, in_maps, core_ids=list(range(E)))

    global last_run_info
    last_run_info = {
        "cap": cap,
        "counts": counts,
        "exec_time_ns": res.exec_time_ns,
        "mean_exec_time_ns": res.mean_exec_time_ns,
        "instructions_and_trace": res.instructions_and_trace,
        "profile_json": res.profile_json,
    }

    # ---- Combine (weighted scatter-add) ----
    out = np.zeros((N, C), dtype=np.float32)
    for e in range(E):
        sel = sels[e]
        out[sel] += res.results[e]["yt"][:, : len(sel)].T.astype(np.float32)
    return out.reshape(B, T, C)


# revision 32
# speedup vs baseline: 1.5601x; 1.0168x over previous
# MoE layer (8 experts, top-2) on 8 TRN2 NeuronCores.
#
# Strategy: expert parallelism (core e owns expert e), per the sharding hint.
#   * Host (control plane): computes gate routing decisions, dispatches
#     ("all-to-all") each token's row to the core(s) owning its top-2 experts,
#     and combines the per-expert partial outputs back into the full output.
#   * Device (data plane): for each core e, computes
#         yT = sigmoid(dlg) * ( W2[e].T @ gelu( W1[e].T @ xT ) )
#     in fp8 (e4m3) DoubleRow perf mode with hi/lo residual compensation.
#
# fp8 DoubleRow: one PE instruction computes lhsT[:,0].T @ rhs[:,0] +
# lhsT[:,1].T @ rhs[:,1] at 0.5 cycles per output row — two fp8 products for
# half the cost of one bf16 k-tile matmul. Every operand O is carried as
# O_hi = fp8(O), O_lo = fp8(O - O_hi), giving ~7 effective mantissa bits.
# Per k-tile pair (A, B) the product (Whi+Wlo)ᵀ(xhi+xlo) is computed as
# three DoubleRow instructions (the lo·lo term is dropped, ~0.07%):
#     main:   [Whi_A, Whi_B] · [xhi_A, xhi_B]
#     crossA: [Whi_A, Wlo_A] · [xlo_A, xhi_A]
#     crossB: [Whi_B, Wlo_B] · [xlo_B, xhi_B]
# = 0.75x the PE cycles of the bf16 kernel with ~0.2% end-to-end error.
#
# Each core's tokens are sorted by their combine weight (descending), and the
# LAST 3 of the 9 token blocks — the ~38% of token-paths with the smallest
# combine weights — drop the x/h (activation) compensation, keeping only the
# weight compensation (main + W_lo cross = 2 products per k-tile, 0.5x bf16).
# Their larger path error (~3.7%) is scaled by small combine weights; the
# measured end-to-end error is ~1.2e-2 against the 2e-2 gate (the numpy
# scheme simulator reproduces the hardware error to 3 decimal places).
#
# Weights are pre-scaled (s1=32, s2=64) so their values sit in e4m3's normal
# range; the scales are undone in the gelu input scale (1/s1) and the
# combine-weight multiply (ce/s2). h = gelu(pre) is split on-chip: the ACT
# engine writes gelu twice (fp8 h_hi and fp32 h), the DVE writes
# h_lo = fp8(h - h_hi).
#
# Dataflow is fully transposed (features on partitions, tokens on the moving
# free dim):
#     phase 1:  hT(F x T)  = W1.T @ xT   (accumulate over C tiles)  -> gelu
#     phase 2:  yT(C x T)  = W2.T @ hT   (accumulate over F tiles)  -> * ce
# W1 (hi+lo fp8, 64 KiB/partition) stays resident in SBUF; W2 streams per
# token block; tokens stream in blocks of 512.

import math

import numpy as np
import ml_dtypes

import concourse.bass as bass
import concourse.mybir as mybir
import concourse.tile as tile
from concourse import bacc
from concourse.bass_utils import run_bass_kernel_spmd

C = 1024          # d_model
F = 4096          # d_ff
E = 8             # experts == cores
P = 128           # SBUF partitions
NTOK = 512        # moving-dim token block (one PSUM bank of fp32)
S1 = 32.0         # W1 pre-scale (W1 ~ N(0, 1/C) -> ~N(0,1))
S2 = 64.0         # W2 pre-scale (W2 ~ N(0, 1/F) -> ~N(0,1))
F8 = mybir.dt.float8e4
BF16 = mybir.dt.bfloat16
F32 = mybir.dt.float32
E4 = ml_dtypes.float8_e4m3
DR = mybir.MatmulPerfMode.DoubleRow

# Filled by kernel() on each call, for the test harness to inspect.
last_run_info: dict = {}

# NEFF-module memo: cap -> compiled Bass module (routing is deterministic in
# the inputs, so repeat calls reuse the same module and its cached NEFF).
_nc_cache: dict = {}


def _block_scheme(b: int, nblk: int) -> str:
    """Compensation scheme per token block (blocks are sorted by combine
    weight, descending): 'F' = full (main + x/W/h crosses), 'W' = weight-only
    (main + W_lo crosses), 'P' = plain (main only)."""
    if b >= nblk - 2:
        return "P"
    if b == nblk - 3:
        return "W"
    return "F"


def _build_ffn(cap: int, ntok: int = NTOK) -> bass.Bass:
    """Per-core expert-FFN kernel (fp8 DoubleRow, hi/lo compensated)."""
    nc = bacc.Bacc()
    CO = C // P   # 8 c-tiles
    FO = F // P   # 32 f-tiles

    # x8: [ci, co, slot, t] with slot 0 = lo, slot 1 = hi (cross pairing
    # needs rhs slots (lo, hi) against lhsT slots (hi, lo)).
    x8 = nc.dram_tensor("x8", [P, CO, 2, cap], F8, kind="ExternalInput")
    # w1: [ci, co, slot, f] with slot 0 = hi, slot 1 = lo.
    w1 = nc.dram_tensor("w1", [P, CO, 2, F], F8, kind="ExternalInput")
    # w2: [fi, co, fo, slot, cc] with slot 0 = hi, slot 1 = lo; each (co)
    # chunk streams as one contiguous 8 KiB-per-partition DMA.
    w2 = nc.dram_tensor("w2", [P, CO, FO, 2, P], F8, kind="ExternalInput")
    dlg = nc.dram_tensor("dlg", [P, cap], F32, kind="ExternalInput")
    yt = nc.dram_tensor("yt", [C, cap], BF16, kind="ExternalOutput")

    yt_r = yt.rearrange("(co ci) t -> ci co t", ci=P)

    with tile.TileContext(nc) as tc:
        with (
            tc.tile_pool(name="wts", bufs=1) as wpool,
            tc.tile_pool(name="w2s", bufs=3) as w2pool,
            tc.tile_pool(name="xts", bufs=2) as xpool,
            tc.tile_pool(name="hts", bufs=1) as hpool,
            tc.tile_pool(name="hfs", bufs=2) as hfpool,
            tc.tile_pool(name="ces", bufs=2) as cepool,
            tc.tile_pool(name="yts", bufs=3) as ypool,
            tc.tile_pool(name="ps", bufs=8, space="PSUM") as pspool,
        ):
            # Block 0's token DMAs are issued BEFORE the w1 load: the DMA
            # queue is FIFO and the first matmul needs x8. The hi plane loads
            # first, then the two w1 chunks the first matmul group reads,
            # then x_lo (needed a few matmuls later), then the rest of w1.
            xt0 = xpool.tile([P, CO, 2, ntok], F8, tag="xt")
            t00 = min(ntok, cap)
            nc.sync.dma_start(xt0[:, :, 1, :t00], x8[:, :, 1, :t00])

            # Resident W1 hi+lo (fp8, 64 KiB/partition), loaded in f-major
            # chunks so phase 1's fo-th psum group only waits for the chunk
            # covering it. W2 streams per token block.
            w1_sb = wpool.tile([P, CO, 2, F], F8, tag="w1")
            FCH = 1024
            for co in range(2):
                nc.sync.dma_start(w1_sb[:, co, :, :FCH], w1[:, co, :, :FCH])
            nc.sync.dma_start(xt0[:, :, 0, :t00], x8[:, :, 0, :t00])
            for co in range(2, CO):
                nc.sync.dma_start(w1_sb[:, co, :, :FCH], w1[:, co, :, :FCH])
            for f0 in range(FCH, F, FCH):
                for co in range(CO):
                    nc.sync.dma_start(
                        w1_sb[:, co, :, f0 : f0 + FCH], w1[:, co, :, f0 : f0 + FCH]
                    )

            nblk = (cap + ntok - 1) // ntok
            # All trailing non-Full blocks (W + plain) merge their phase 2
            # into ONE pass: each streamed interleaved W2 chunk feeds every
            # trailing block's psum group, so the chunk cadence (~6us) stays
            # far above its ~2.8us transfer time — the separate W-block and
            # short-last-block phase-2 windows would each starve the DMA.
            # The trailing blocks only write h_hi planes: two share the
            # 2-slot ht tile, the third uses a 16 KiB hi-only tile.
            trail = [b for b in range(nblk) if _block_scheme(b, nblk) != "F"]
            # Phase-1 groups of the next block emitted BEFORE the pending
            # phase 2 (activations deferred after it): the PE chews on them
            # while the previous block's gelu/h_lo tail drains, instead of
            # stalling ~1.2us at every phase1 -> phase2 boundary.
            PK = 3

            def p1_mm(c, fo):
                """Emit phase-1 matmul group fo of block c; return its psum."""
                tn, mode, xt_t = c["tn"], c["mode"], c["xt"]
                fsl = slice(fo * P, (fo + 1) * P)
                ps = pspool.tile([P, ntok], F32, tag="ps")
                for j in range(CO // 2):
                    nc.tensor.matmul(
                        ps[:, :tn],
                        w1_sb[:, 2 * j : 2 * j + 2, 0, fsl],
                        xt_t[:, 2 * j : 2 * j + 2, 1, :tn],
                        start=(j == 0),
                        stop=(mode == "P" and j == CO // 2 - 1),
                        perf_mode=DR,
                    )
                if mode == "F":
                    for co in range(CO):
                        nc.tensor.matmul(
                            ps[:, :tn],
                            w1_sb[:, co, :, fsl],
                            xt_t[:, co, :, :tn],
                            start=False, stop=(co == CO - 1), perf_mode=DR,
                        )
                elif mode == "W":
                    for j in range(CO // 2):
                        nc.tensor.matmul(
                            ps[:, :tn],
                            w1_sb[:, 2 * j : 2 * j + 2, 1, fsl],
                            xt_t[:, 2 * j : 2 * j + 2, 1, :tn],
                            start=False, stop=(j == CO // 2 - 1),
                            perf_mode=DR,
                        )
                return ps

            def p1_act(c, fo, ps):
                """h_hi = fp8(gelu(ps/S1)); for full blocks also the fp32
                gelu (same ACT table, bit-identical inputs) and
                h_lo = fp8(h - h_hi)."""
                tn, ht_t, h_slot = c["tn"], c["ht"], c["slot"]
                nc.scalar.activation(
                    ht_t[:, fo, h_slot, :tn], ps[:, :tn],
                    mybir.ActivationFunctionType.Gelu, scale=1.0 / S1,
                )
                if c["mode"] == "F":
                    hf_t = hfpool.tile([P, ntok], F32, tag="hf")
                    nc.scalar.activation(
                        hf_t[:, :tn], ps[:, :tn],
                        mybir.ActivationFunctionType.Gelu, scale=1.0 / S1,
                    )
                    nc.vector.tensor_tensor(
                        ht_t[:, fo, 0, :tn], hf_t[:, :tn],
                        ht_t[:, fo, 1, :tn], mybir.AluOpType.subtract,
                    )

            def phase2(groups, filler=()):
                """yT = (ce/S2) * (S2 * W2.T @ hT). Each streamed interleaved
                W2 chunk feeds every group's psum accumulation. `filler`
                (ctx, fo) phase-1 groups are spread between the co chunks to
                soak up PE time while their ACT work overlaps this phase."""
                for co in range(CO):
                    for fc, ffo in filler[
                        co * len(filler) // CO : (co + 1) * len(filler) // CO
                    ]:
                        p1_act(fc, ffo, p1_mm(fc, ffo))
                    w2_t = w2pool.tile([P, FO, 2, P], F8, tag="w2s")
                    nc.sync.dma_start(w2_t[:], w2[:, co, :, :, :])
                    for g in groups:
                        g_tn, g_t0, g_ce = g["tn"], g["t0"], g["ce"]
                        ht_t, g_slot, mode = g["ht"], g["slot"], g["mode"]
                        ps2 = pspool.tile([P, ntok], F32, tag="ps")
                        for j in range(FO // 2):
                            nc.tensor.matmul(
                                ps2[:, :g_tn],
                                w2_t[:, 2 * j : 2 * j + 2, 0, :],
                                ht_t[:, 2 * j : 2 * j + 2, g_slot, :g_tn],
                                start=(j == 0),
                                stop=(mode == "P" and j == FO // 2 - 1),
                                perf_mode=DR,
                            )
                        if mode == "F":
                            for fo in range(FO):
                                nc.tensor.matmul(
                                    ps2[:, :g_tn],
                                    w2_t[:, fo, :, :],
                                    ht_t[:, fo, :, :g_tn],
                                    start=False, stop=(fo == FO - 1),
                                    perf_mode=DR,
                                )
                        elif mode == "W":
                            for j in range(FO // 2):
                                nc.tensor.matmul(
                                    ps2[:, :g_tn],
                                    w2_t[:, 2 * j : 2 * j + 2, 1, :],
                                    ht_t[:, 2 * j : 2 * j + 2, g_slot, :g_tn],
                                    start=False, stop=(j == FO // 2 - 1),
                                    perf_mode=DR,
                                )
                        y_t = ypool.tile([P, ntok], BF16, tag="y")
                        nc.vector.tensor_tensor(
                            y_t[:, :g_tn], ps2[:, :g_tn], g_ce[:, :g_tn],
                            mybir.AluOpType.mult,
                        )
                        nc.sync.dma_start(
                            yt_r[:, co, g_t0 : g_t0 + g_tn], y_t[:, :g_tn]
                        )

            ht_trail = None          # 2-slot ht shared by the first two
            pend = None

            def make_ctx(b):
                """Emit block b's x8/dlg loads + ce ops; return its context."""
                nonlocal ht_trail
                t0 = b * ntok
                tn = min(ntok, cap - t0)
                mode = _block_scheme(b, nblk)
                if b == 0:
                    xt_t = xt0
                else:
                    xt_t = xpool.tile([P, CO, 2, ntok], F8, tag="xt")
                    if mode == "F":
                        nc.sync.dma_start(
                            xt_t[:, :, :, :tn], x8[:, :, :, t0 : t0 + tn]
                        )
                    else:
                        # W-only/plain blocks never read the x_lo plane.
                        nc.sync.dma_start(
                            xt_t[:, :, 1, :tn], x8[:, :, 1, t0 : t0 + tn]
                        )
                # Combine weight ce = sigmoid(dlg)/S2 = (0.5*tanh(dlg/2)+0.5)/S2
                # (tanh shares an ACT table with gelu; sigmoid does not). The
                # 1/S2 undoes the W2 pre-scale.
                dlg_t = cepool.tile([P, ntok], F32, tag="dlg", bufs=1)
                nc.sync.dma_start(dlg_t[:, :tn], dlg[:, t0 : t0 + tn])
                ce_t = cepool.tile([P, ntok], F32, tag="ce", bufs=3)
                nc.scalar.activation(
                    ce_t[:, :tn], dlg_t[:, :tn],
                    mybir.ActivationFunctionType.Tanh, scale=0.5,
                )
                nc.vector.tensor_scalar(
                    ce_t[:, :tn], ce_t[:, :tn], 0.5 / S2, 0.5 / S2,
                    mybir.AluOpType.mult, mybir.AluOpType.add,
                )
                if b in trail:
                    ti = trail.index(b)
                    if ti < 2:
                        if ht_trail is None:
                            ht_trail = hpool.tile([P, FO, 2, ntok], F8, tag="ht")
                        ht_t, h_slot = ht_trail, ti
                    else:
                        ht_t = hpool.tile([P, FO, 1, ntok], F8, tag="htp")
                        h_slot = 0
                else:
                    ht_t = hpool.tile([P, FO, 2, ntok], F8, tag="ht")
                    h_slot = 1
                return {
                    "t0": t0, "tn": tn, "mode": mode, "xt": xt_t,
                    "ht": ht_t, "slot": h_slot, "ce": ce_t,
                }

            # Full blocks: phase 1, then (prefix-pipelined) phase 2.
            for b in range(nblk - len(trail)):
                c = make_ctx(b)
                held = []
                if pend is not None:
                    # Prefix-pipeline across Full-block boundaries: their
                    # gelu/h_lo tail is the long one.
                    for fo in range(min(PK, FO)):
                        held.append((fo, p1_mm(c, fo)))
                    phase2(pend)
                    pend = None
                for fo, ps in held:
                    p1_act(c, fo, ps)
                for fo in range(len(held), FO):
                    p1_act(c, fo, p1_mm(c, fo))
                pend = [c]

            # Trailing W/plain blocks. Their plain phase-1 groups are
            # ACT-bound (4 matmuls = 428 ns vs ~620 ns of gelu), so
            # interleave them with PE-dense work instead of running them
            # back-to-back: the last block's groups (whose 16 KiB htp tile
            # is free early) slot into the last full block's phase 2, and
            # the second-trailing block's groups pair with the W block's
            # PE-bound ones. One merged phase 2 (a single W2 chunk stream)
            # then serves all trailing blocks.
            if trail:
                # The last trail block's ctx is created FIRST so its xt tile
                # lands in a ring slot whose previous user (an already-emitted
                # phase 1) has finished — its filler groups run inside the
                # last full block's phase 2, before the other trail phase 1s.
                if len(trail) == 3:
                    c_last = make_ctx(trail[2])
                    ctxs = [make_ctx(trail[0]), make_ctx(trail[1]), c_last]
                    filler = [(c_last, fo) for fo in range(FO)]
                else:
                    ctxs = [make_ctx(b) for b in trail]
                    filler = []
                if pend is not None:
                    phase2(pend, filler=filler)
                    pend = None
                else:
                    for fc, ffo in filler:
                        p1_act(fc, ffo, p1_mm(fc, ffo))
                for fo in range(FO):
                    for c in ctxs[: min(2, len(ctxs))]:
                        p1_act(c, fo, p1_mm(c, fo))
                phase2(ctxs)
            elif pend is not None:
                phase2(pend)

    # bacc passes: register allocation, and crucially generate_event_semaphores,
    # which splits multi-wait sync conditions (HW allows 1 wait per instruction).
    nc.compile()

    # Guard: the Tile allocator believes SBUF is 224 KiB/partition (the ISA
    # constant), but exceeding ~192 KiB crashes the TRN2 exec unit. Keep a
    # hard ceiling so overflows fail at build time, not on silicon.
    hw = 0
    for alloc in nc.to_json()["functions"][0]["allocations"]:
        for ml in alloc.get("memorylocations") or []:
            if ml.get("type") == "SB":
                hw = max(hw, ml["addr"] + ml["dims"][1])
    assert hw <= 184 * 1024, f"SBUF high-water {hw / 1024:.1f} KiB exceeds 184 KiB"
    return nc


def _gate_jax_cpu(xf: np.ndarray, Wg: np.ndarray):
    """Reproduce the reference's gate bit-exactly: fp32 matmul + lax.top_k
    on the jax CPU backend (including its tie-breaking). Falls back to a
    numpy gate (correct except possibly on exact fp32 knife-edge ties) if
    jax is unavailable."""
    try:
        import jax

        cpu = jax.devices("cpu")[0]
        with jax.default_device(cpu):
            logits = jax.device_put(xf, cpu) @ jax.device_put(Wg, cpu)
            tv, ti = jax.lax.top_k(logits, 2)
            return np.asarray(ti), np.asarray(tv)
    except Exception:
        logits = xf @ Wg
        part = np.argpartition(-logits, 1, axis=1)[:, :2]
        pv = np.take_along_axis(logits, part, axis=1)
        order = np.argsort(-pv, axis=1, kind="stable")
        ti = np.take_along_axis(part, order, axis=1)
        tv = np.take_along_axis(logits, ti, axis=1)
        return ti, tv


def _split8(a: np.ndarray):
    """hi/lo e4m3 residual split of a float32 array."""
    hi = a.astype(E4)
    lo = (a - hi.astype(np.float32)).astype(E4)
    return hi, lo


def kernel(x, Wg, W1, W2):
    x = np.asarray(x, dtype=np.float32)
    Wg = np.asarray(Wg, dtype=np.float32)
    W1 = np.asarray(W1, dtype=np.float32)
    W2 = np.asarray(W2, dtype=np.float32)

    B, T, _ = x.shape
    N = B * T
    xf = x.reshape(N, C)
    CO, FO = C // P, F // P

    # ---- Gate + routing (control plane) ----
    # Routing decisions are knife-edge sensitive: compute the gate with the
    # same jax-on-CPU ops the reference uses so the top-2 selection matches
    # it bit-for-bit.
    top2, tv = _gate_jax_cpu(xf, Wg)                        # (N, 2) ids / logits

    # Softmax weights for the sort: own = weight of the owning expert.
    wsm = np.exp(tv - tv.max(1, keepdims=True))
    wsm = wsm / wsm.sum(1, keepdims=True)

    sels = []
    counts = []
    for e in range(E):
        sel = np.nonzero((top2 == e).any(axis=1))[0]
        # Sort descending by this expert's combine weight so the trailing
        # (W-only compensated) blocks hold the lowest-stakes token paths.
        own_w = np.where(top2[sel, 0] == e, wsm[sel, 0], wsm[sel, 1])
        sel = sel[np.argsort(-own_w, kind="stable")]
        sels.append(sel)
        counts.append(len(sel))
    # cap needs no partition alignment — tokens are the free dim everywhere.
    # Round to even so the DoubleRow moving pair stays aligned.
    cap = max(NTOK, math.ceil(max(counts) / 2) * 2)

    # ---- Token dispatch (all-to-all equivalent) ----
    in_maps = []
    for e in range(E):
        sel = sels[e]
        cnt = len(sel)
        row = top2[sel]
        tvr = tv[sel]
        own = np.where(row[:, 0] == e, tvr[:, 0], tvr[:, 1])
        other = np.where(row[:, 0] == e, tvr[:, 1], tvr[:, 0])

        # x8: [ci, co, slot(0=lo,1=hi), t]
        xe = np.zeros((P, CO, 2, cap), dtype=E4)
        xt = xf[sel].T.reshape(CO, P, cnt).transpose(1, 0, 2)  # (ci, co, t)
        xhi, xlo = _split8(xt)
        xe[:, :, 0, :cnt] = xlo
        xe[:, :, 1, :cnt] = xhi

        dlg_v = np.full((cap,), -60.0, dtype=np.float32)
        dlg_v[:cnt] = own - other
        dlg_b = np.ascontiguousarray(
            np.broadcast_to(dlg_v[None, :], (P, cap)), dtype=np.float32
        )

        # w1: [ci, co, slot(0=hi,1=lo), f]
        w1t = (W1[e] * S1).reshape(CO, P, F).transpose(1, 0, 2)  # (ci, co, f)
        w1hi, w1lo = _split8(w1t)
        w1e = np.stack([w1hi, w1lo], axis=2)

        # w2: [fi, co, fo, slot(0=hi,1=lo), cc]
        w2t = (W2[e] * S2).reshape(FO, P, CO, P).transpose(1, 2, 0, 3)
        w2hi, w2lo = _split8(w2t)                      # (fi, co, fo, cc)
        w2e = np.stack([w2hi, w2lo], axis=3)

        in_maps.append(
            {
                "x8": np.ascontiguousarray(xe),
                "w1": np.ascontiguousarray(w1e),
                "w2": np.ascontiguousarray(w2e),
                "dlg": dlg_b,
            }
        )

    # ---- Expert FFN on the 8 NeuronCores ----
    nc = _nc_cache.get(cap)
    if nc is None:
        nc = _nc_cache[cap] = _build_ffn(cap)
    res = run_bass_kernel_spmd(nc, in_maps, core_ids=list(range(E)))

    global last_run_info
    last_run_info = {
        "cap": cap,
        "counts": counts,
        "exec_time_ns": res.exec_time_ns,
        "mean_exec_time_ns": res.mean_exec_time_ns,
        "instructions_and_trace": res.instructions_and_trace,
        "profile_json": res.profile_json,
    }

    # ---- Combine (weighted scatter-add) ----
    out = np.zeros((N, C), dtype=np.float32)
    for e in range(E):
        sel = sels[e]
        out[sel] += res.results[e]["yt"][:, : len(sel)].T.astype(np.float32)
    return out.reshape(B, T, C)
